# revision 1
# baseline (speedup 1.0000x reference)
"""DeepSet segment-reduce kernel for 8 Trainium2 NeuronCores.

Math (equivalent to the reference, using linearity of segment_sum):
    r      = relu(x @ W1 + b1)                      # per-node, on device
    sums_r = segment_sum(r)                         # [B, HID]
    mean_r = sums_r / max(counts, 1)                # counts via host bincount
    hid    = mean_r @ W2 + b2                       # tiny tail, on device
    out    = relu(hid @ W3 + b3) @ W4 + b4          # tiny tail, on device

Phase 1 (8 cores, data-parallel over nodes): each core's ~N/8 nodes are
split into two contiguous halves packed on SBUF partition halves (features
of half A in partitions 0..63, half B in 64..127, bf16), so DMA runs at
full 128-partition width.  Each half's segment runs are zero-padded to
multiples of 1024 columns, so every 1024-column superblock belongs to one
segment.  Matmuls use K=128 with zero-padded weights wzA=[[W1],[0]],
wzB=[[0],[W1]] — full-array matmuls keep the PE's HAM activity monitor
un-throttled at 2.4 GHz (K=64 matmuls leave it stuck at 1.2 GHz), and both
weight tiles live at PE tile position (0,0) (bf16 LDWEIGHTS at row tile 64
is broken in hardware).  Per superblock and half: 2 matmuls fill a 2-bank
PSUM tile, then ONE fused relu+bias+sum produces the [128,1] partial:
half A on the Scalar engine (ACT Relu, bias, accum_out), half B on the
Vector engine (tensor_scalar max(-b1)/add-reduce; the sum is off by
SB*b1, restored on the host).  With 4 PSUM tiles in flight both reduce
engines run continuously while the PE refills the other tiles.

The host routes superblock partials to segments, removes the pad columns'
relu(b1) contribution, applies the mean, and a second tiny NEFF runs the
rho MLP in bf16.  Segments with zero nodes are fixed up on the host
(reference gives relu(b3) @ W4 + b4 there).
"""

import os
import sys

for _p in ("/opt/trn_rl_repo",):
    if os.path.isdir(_p) and _p not in sys.path:
        sys.path.append(_p)

import numpy as np
import ml_dtypes

import concourse.bass as bass
import concourse.tile as tile
from concourse import bacc, mybir
from concourse.bass_utils import run_bass_kernel_spmd

F32 = mybir.dt.float32
BF16 = mybir.dt.bfloat16

NCORES = 8
TILE = 512
SB = 1024            # superblock columns (2 PSUM banks; 4 tiles in flight)
SB_PER_CHUNK = 8     # superblocks per DMA chunk
NSEG = 1024
ODIM = 16


def _pad_runs(ids, lo, ch):
    """Segment runs of a sorted id slice, padded to SB multiples.
    Returns (src indices with -1 pads, seg id per superblock)."""
    uniq, starts = np.unique(ids, return_index=True)
    ends = np.append(starts[1:], ch)
    seg_of_sb = []
    src_parts = []
    for k in range(len(uniq)):
        L = int(ends[k] - starts[k])
        T = -(-L // SB)
        arr = np.full(T * SB, -1, dtype=np.int64)
        arr[:L] = lo + starts[k] + np.arange(L)
        src_parts.append(arr)
        seg_of_sb += [int(uniq[k])] * T
    src = np.concatenate(src_parts) if src_parts else np.empty(0, np.int64)
    return src, seg_of_sb


def _host_prep(x, x_batch, ncores=NCORES):
    N = x.shape[0]
    assert N % (2 * ncores) == 0
    ch = N // (2 * ncores)          # nodes per half
    xb = np.asarray(x_batch)

    counts = np.bincount(xb, minlength=NSEG).astype(np.float64)

    halves = []                      # (src, seg_of_sb) per (core, half)
    n_sb = 0
    for c in range(ncores):
        for h in range(2):
            lo = (2 * c + h) * ch
            src, seg_of_sb = _pad_runs(xb[lo:lo + ch], lo, ch)
            halves.append((src, seg_of_sb))
            n_sb = max(n_sb, len(seg_of_sb))

    cols = n_sb * SB
    padcount = np.zeros(NSEG, dtype=np.float64)
    xts = []
    seg_a, seg_d = [], []
    for c in range(ncores):
        xt = np.zeros((128, cols), dtype=ml_dtypes.bfloat16)
        for h in range(2):
            src, seg_of_sb = halves[2 * c + h]
            if len(src) < cols:
                src = np.concatenate([src, np.full(cols - len(src), -1, np.int64)])
            mask = src >= 0
            gath = np.zeros((cols, 64), dtype=np.float32)
            gath[mask] = x[src[mask]]
            xt[64 * h:64 * h + 64, :] = gath.T.astype(ml_dtypes.bfloat16)
            if seg_of_sb:
                seg_arr = np.array(seg_of_sb, dtype=np.int64)
                real = mask[:len(seg_arr) * SB].reshape(-1, SB).sum(axis=1)
                np.add.at(padcount, seg_arr, SB - real)
            (seg_a if h == 0 else seg_d).append(seg_of_sb)
        xts.append(xt)

    meta = dict(n_sb=n_sb, cols=cols, counts=counts, padcount=padcount,
                seg_a=seg_a, seg_d=seg_d, ncores=ncores)
    return xts, meta


def _build_phase1(n_sb, cols, ncores=NCORES):
    nc = bacc.Bacc("TRN2", target_bir_lowering=False, debug=False,
                   num_devices=ncores)
    xt_d = nc.dram_tensor("xt", [128, cols], BF16, kind="ExternalInput").ap()
    wza_d = nc.dram_tensor("wza", [128, 128], BF16, kind="ExternalInput").ap()
    wzb_d = nc.dram_tensor("wzb", [128, 128], BF16, kind="ExternalInput").ap()
    b1_d = nc.dram_tensor("b1", [128, 1], F32, kind="ExternalInput").ap()
    nb1_d = nc.dram_tensor("nb1", [128, 1], F32, kind="ExternalInput").ap()
    sa_d = nc.dram_tensor("s_act", [128, n_sb], F32, kind="ExternalOutput").ap()
    sd_d = nc.dram_tensor("s_dve", [128, n_sb], F32, kind="ExternalOutput").ap()

    CH = SB_PER_CHUNK * SB

    with tile.TileContext(nc) as tc:
        with tc.tile_pool(name="const", bufs=1) as cpool, \
             tc.tile_pool(name="xin", bufs=3) as xpool, \
             tc.tile_pool(name="tr", bufs=1) as trpool, \
             tc.tile_pool(name="ps", bufs=2, space="PSUM") as pspool:

            wza = cpool.tile([128, 128], BF16)
            nc.sync.dma_start(wza[:], wza_d[:])
            wzb = cpool.tile([128, 128], BF16)
            nc.sync.dma_start(wzb[:], wzb_d[:])
            b1t = cpool.tile([128, 1], F32)
            nc.sync.dma_start(b1t[:], b1_d[:])
            nb1t = cpool.tile([128, 1], F32)
            nc.sync.dma_start(nb1t[:], nb1_d[:])
            S_a = cpool.tile([128, n_sb], F32)
            nc.vector.memset(S_a[:], 0.0)
            S_d = cpool.tile([128, n_sb], F32)
            nc.vector.memset(S_d[:], 0.0)

            xtile = None
            for sb in range(n_sb):
                if sb % SB_PER_CHUNK == 0:
                    # one SBUF chunk, filled by per-superblock DMAs so the
                    # first matmuls start after ~256 KB instead of ~2 MB
                    xtile = xpool.tile([128, CH], BF16, tag="x")
                    for j in range(min(SB_PER_CHUNK, n_sb - sb)):
                        lo = (sb + j) * SB
                        nc.sync.dma_start(xtile[:, j * SB:(j + 1) * SB],
                                          xt_d[:, lo:lo + SB])
                base = (sb % SB_PER_CHUNK) * SB
                psa = pspool.tile([128, SB], F32, tag="psa")
                psb = pspool.tile([128, SB], F32, tag="psb")
                for t in range(SB // TILE):
                    off = base + t * TILE
                    nc.tensor.matmul(
                        psa[:, t * TILE:t * TILE + TILE], lhsT=wza[:],
                        rhs=xtile[:, off:off + TILE], start=True, stop=True)
                for t in range(SB // TILE):
                    off = base + t * TILE
                    nc.tensor.matmul(
                        psb[:, t * TILE:t * TILE + TILE], lhsT=wzb[:],
                        rhs=xtile[:, off:off + TILE], start=True, stop=True)
                trash_a = trpool.tile([128, SB], BF16, tag="ta")
                nc.scalar.activation(
                    out=trash_a[:], in_=psa[:],
                    func=mybir.ActivationFunctionType.Relu,
                    bias=b1t[:, 0:1],
                    accum_out=S_a[:, sb:sb + 1])
                # accum_out = add-reduce of max(psum, -b1)
                #           = sum(relu(psum + b1)) - SB*b1  (host adds it back)
                trash_d = trpool.tile([128, SB], BF16, tag="td")
                nc.vector.tensor_scalar(
                    out=trash_d[:], in0=psb[:],
                    scalar1=nb1t[:, 0:1], scalar2=0.0,
                    op0=mybir.AluOpType.max, op1=mybir.AluOpType.add,
                    accum_out=S_d[:, sb:sb + 1])

            nc.sync.dma_start(sa_d[:], S_a[:])
            nc.sync.dma_start(sd_d[:], S_d[:])

    nc.compile()
    return nc


def _build_phase2():
    nc = bacc.Bacc("TRN2", target_bir_lowering=False, debug=False, num_devices=1)
    mean_d = nc.dram_tensor("mean", [128, NSEG], BF16, kind="ExternalInput").ap()
    w2_d = nc.dram_tensor("w2", [128, 128], BF16, kind="ExternalInput").ap()
    w3_d = nc.dram_tensor("w3", [128, 128], BF16, kind="ExternalInput").ap()
    w4_d = nc.dram_tensor("w4", [128, ODIM], BF16, kind="ExternalInput").ap()
    b2_d = nc.dram_tensor("b2", [128, 1], F32, kind="ExternalInput").ap()
    b3_d = nc.dram_tensor("b3", [128, 1], F32, kind="ExternalInput").ap()
    b4_d = nc.dram_tensor("b4", [ODIM, 1], F32, kind="ExternalInput").ap()
    out_d = nc.dram_tensor("out_t", [ODIM, NSEG], F32, kind="ExternalOutput").ap()

    with tile.TileContext(nc) as tc:
        with tc.tile_pool(name="sb", bufs=1) as pool, \
             tc.tile_pool(name="ps", bufs=2, space="PSUM") as psp:
            mean = pool.tile([128, NSEG], BF16)
            nc.sync.dma_start(mean[:], mean_d[:])
            w2 = pool.tile([128, 128], BF16)
            nc.sync.dma_start(w2[:], w2_d[:])
            w3 = pool.tile([128, 128], BF16)
            nc.sync.dma_start(w3[:], w3_d[:])
            w4 = pool.tile([128, ODIM], BF16)
            nc.sync.dma_start(w4[:], w4_d[:])
            b2 = pool.tile([128, 1], F32)
            nc.sync.dma_start(b2[:], b2_d[:])
            b3 = pool.tile([128, 1], F32)
            nc.sync.dma_start(b3[:], b3_d[:])
            b4 = pool.tile([ODIM, 1], F32)
            nc.sync.dma_start(b4[:], b4_d[:])

            hid = pool.tile([128, NSEG], BF16)
            t3 = pool.tile([128, NSEG], BF16)
            ot = pool.tile([ODIM, NSEG], F32)
            # all bias/relu on DVE tensor_scalar: avoids the ~2.7us ACT
            # table load entirely (no Scalar instructions in this NEFF)
            for j in range(NSEG // 512):
                sl = slice(512 * j, 512 * j + 512)
                p2 = psp.tile([128, 512], F32, tag="p")
                nc.tensor.matmul(p2[:], lhsT=w2[:], rhs=mean[:, sl],
                                 start=True, stop=True)
                nc.vector.tensor_scalar(out=hid[:, sl], in0=p2[:],
                                        scalar1=b2[:, 0:1], scalar2=None,
                                        op0=mybir.AluOpType.add)
            for j in range(NSEG // 512):
                sl = slice(512 * j, 512 * j + 512)
                p3 = psp.tile([128, 512], F32, tag="p")
                nc.tensor.matmul(p3[:], lhsT=w3[:], rhs=hid[:, sl],
                                 start=True, stop=True)
                nc.vector.tensor_scalar(out=t3[:, sl], in0=p3[:],
                                        scalar1=b3[:, 0:1], scalar2=0.0,
                                        op0=mybir.AluOpType.add,
                                        op1=mybir.AluOpType.max)
            for j in range(NSEG // 512):
                sl = slice(512 * j, 512 * j + 512)
                p4f = psp.tile([128, 512], F32, tag="p")
                p4 = p4f[:ODIM, :]
                nc.tensor.matmul(p4, lhsT=w4[:], rhs=t3[:, sl],
                                 start=True, stop=True)
                nc.vector.tensor_scalar(out=ot[:, sl], in0=p4,
                                        scalar1=b4[:, 0:1], scalar2=None,
                                        op0=mybir.AluOpType.add)
            nc.sync.dma_start(out_d[:], ot[:])
    nc.compile()
    return nc


def run(inputs, ncores=NCORES, trace=False):
    x = np.asarray(inputs["x"], dtype=np.float32)
    xb = np.asarray(inputs["x_batch"])
    W1 = np.asarray(inputs["W1"], dtype=np.float32)
    b1 = np.asarray(inputs["b1"], dtype=np.float32)

    xts, meta = _host_prep(x, xb, ncores=ncores)
    n_sb, cols = meta["n_sb"], meta["cols"]

    wza = np.zeros((128, 128), dtype=np.float32)
    wza[0:64, :] = W1
    wzb = np.zeros((128, 128), dtype=np.float32)
    wzb[64:128, :] = W1
    wza = wza.astype(ml_dtypes.bfloat16)
    wzb = wzb.astype(ml_dtypes.bfloat16)
    b1c = np.ascontiguousarray(b1, np.float32).reshape(128, 1)
    nb1c = np.ascontiguousarray(-b1, np.float32).reshape(128, 1)
    in_maps = [dict(xt=xts[c], wza=wza, wzb=wzb, b1=b1c, nb1=nb1c)
               for c in range(ncores)]

    nc1 = _build_phase1(n_sb, cols, ncores=ncores)
    res1 = run_bass_kernel_spmd(nc1, in_maps, core_ids=list(range(ncores)),
                                trace=trace)

    # host: route superblock partials to segments, 8-core combine.
    # Vector-path sums are sum(max(psum,-b1)) = sum(relu(psum+b1)) - SB*b1.
    b1f = b1.astype(np.float64)
    gsums = np.zeros((NSEG, 128), dtype=np.float64)
    for c in range(ncores):
        Sa = res1.results[c]["s_act"].astype(np.float64)   # [128, n_sb]
        Sd = res1.results[c]["s_dve"].astype(np.float64)
        seg = np.array(meta["seg_a"][c], dtype=np.int64)
        if len(seg):
            np.add.at(gsums, seg, Sa.T[:len(seg)])
        seg = np.array(meta["seg_d"][c], dtype=np.int64)
        if len(seg):
            np.add.at(gsums, seg, Sd.T[:len(seg)] + SB * b1f[None, :])
    # remove the relu(b1) contribution of zero-pad columns
    gsums -= np.maximum(b1, 0.0)[None, :].astype(np.float64) * meta["padcount"][:, None]

    counts = meta["counts"]
    mean = gsums / np.maximum(counts, 1.0)[:, None]                  # [NSEG,128]

    p2_ins = [dict(
        mean=np.ascontiguousarray(mean.T.astype(ml_dtypes.bfloat16)),
        w2=np.ascontiguousarray(inputs["W2"], np.float32).astype(ml_dtypes.bfloat16),
        w3=np.ascontiguousarray(inputs["W3"], np.float32).astype(ml_dtypes.bfloat16),
        w4=np.ascontiguousarray(inputs["W4"], np.float32).astype(ml_dtypes.bfloat16),
        b2=np.ascontiguousarray(inputs["b2"], np.float32).reshape(128, 1),
        b3=np.ascontiguousarray(inputs["b3"], np.float32).reshape(128, 1),
        b4=np.ascontiguousarray(inputs["b4"], np.float32).reshape(ODIM, 1),
    )]
    nc2 = _build_phase2()
    res2 = run_bass_kernel_spmd(nc2, p2_ins, core_ids=[0], trace=trace)
    out = np.ascontiguousarray(res2.results[0]["out_t"].T).astype(np.float32)

    # segments with no nodes: reference's hid is 0 (not b2), so
    # out = relu(b3) @ W4 + b4 exactly
    empty = counts == 0
    if empty.any():
        row = (np.maximum(np.asarray(inputs["b3"], np.float64), 0.0)
               @ np.asarray(inputs["W4"], np.float64)
               + np.asarray(inputs["b4"], np.float64))
        out[empty] = row.astype(np.float32)
    return out, res1, res2


def kernel(**inputs):
    inputs = {k: np.asarray(v) for k, v in inputs.items()}
    out, _, _ = run(inputs)
    return out


if __name__ == "__main__":
    rng = np.random.default_rng(0)
    N, D, HN, B = 8 * 32 * SB, 64, 128, 64
    x = rng.standard_normal((N, D), dtype=np.float32)
    xb = np.sort(rng.integers(0, B, N).astype(np.int32))
    W1 = (rng.standard_normal((D, HN)) / 8).astype(np.float32)
    W2 = (rng.standard_normal((HN, HN)) / 11.3).astype(np.float32)
    W3 = (rng.standard_normal((HN, HN)) / 11.3).astype(np.float32)
    W4 = (rng.standard_normal((HN, ODIM)) / 11.3).astype(np.float32)
    b1 = rng.standard_normal(HN).astype(np.float32) * 0.1
    b2 = rng.standard_normal(HN).astype(np.float32) * 0.1
    b3 = rng.standard_normal(HN).astype(np.float32) * 0.1
    b4 = rng.standard_normal(ODIM).astype(np.float32) * 0.1
    ins = dict(x=x, x_batch=xb, W1=W1, b1=b1, W2=W2, b2=b2, W3=W3, b3=b3,
               W4=W4, b4=b4)
    out = kernel(**ins)

    h = np.maximum(x @ W1 + b1, 0) @ W2 + b2
    sums = np.zeros((1024, HN), dtype=np.float64)
    np.add.at(sums, xb, h.astype(np.float64))
    cnt = np.bincount(xb, minlength=1024).astype(np.float64)
    mean = sums / np.maximum(cnt, 1)[:, None]
    ref = (np.maximum(mean @ W3 + b3, 0) @ W4 + b4).astype(np.float32)
    num = np.linalg.norm(out - ref)
    den = np.linalg.norm(ref)
    print("Relative error:", num / den)



# revision 2
# speedup vs baseline: 1.2296x; 1.2296x over previous
"""DeepSet segment-reduce kernel for 8 Trainium2 NeuronCores.

Single fused NEFF per core (SPMD over 8 cores):
  1. phi matmul: z = x @ W1 on the PE (fp8 inputs, K=128 with the two
     node-halves packed on SBUF partition halves and zero-padded fp8
     weights wza=[[W1],[0]], wzb=[[0],[W1]]).
  2. fused relu+window-sum: per 1024-column window, one ACT activation
     (half A) or DVE tensor_scalar (half B) with accum_out writes the
     [128,1] window partial into P.  Windows are segment-aligned via
     per-run padding (~8% pad), so each window belongs to one segment.
  3. bias/pad correction: a tiny K=32 matmul builds the exact
     correction (pad columns contribute relu(b1) on the ACT path and
     relu(-b1) on the DVE path, real DVE columns are offset by -b1),
     added to P -> Pc.
  4. rho tail on device: routing matmuls fold the window->segment map,
     the 1/count mean scaling and G = W2@W3 into three K=128 matmuls
     (lhsT = Pc chunk), a PE transpose restores [hid, seg] orientation,
     ACT applies relu(. + c3) with c3 = b2@W3 + b3, and a final W4
     matmul + b4 bias produces out rows for 128 device-owned segments
     per core.
  5. host: segments not fully owned by one core's 128 slots (core
     boundaries, leftovers, empties) are recomputed exactly from the
     exported Pc partials; everything is assembled into [1024, 16].
"""

import os
import sys

for _p in ("/opt/trn_rl_repo",):
    if os.path.isdir(_p) and _p not in sys.path:
        sys.path.append(_p)

import numpy as np
import ml_dtypes

import concourse.bass as bass
import concourse.tile as tile
from concourse import bacc, mybir
from concourse.bass_utils import run_bass_kernel_spmd

F32 = mybir.dt.float32
BF16 = mybir.dt.bfloat16
FP8 = mybir.dt.float8e4
F8NP = ml_dtypes.float8_e4m3

NCORES = 8
SB = 1024            # window columns (one [128,1024] fp32 psum tile pair)
SB_PER_CHUNK = 8
NSEG = 1024
ODIM = 16
NSLOT = 128          # device-owned segments per core


def _pad_runs(ids, lo, ch):
    """Segment runs of a sorted id slice, padded to SB multiples.
    Returns (src indices with -1 pads, seg id per window)."""
    uniq, starts = np.unique(ids, return_index=True)
    ends = np.append(starts[1:], ch)
    seg_of_w = []
    src_parts = []
    for k in range(len(uniq)):
        L = int(ends[k] - starts[k])
        T = -(-L // SB)
        arr = np.full(T * SB, -1, dtype=np.int64)
        arr[:L] = lo + starts[k] + np.arange(L)
        src_parts.append(arr)
        seg_of_w += [int(uniq[k])] * T
    src = np.concatenate(src_parts) if src_parts else np.empty(0, np.int64)
    return src, seg_of_w


def _host_prep(x, x_batch, ncores=NCORES):
    N = x.shape[0]
    assert N % (2 * ncores) == 0
    ch = N // (2 * ncores)          # nodes per half
    xb = np.asarray(x_batch)

    counts = np.bincount(xb, minlength=NSEG).astype(np.float64)

    halves = []                      # (src, seg_of_w) per (core, half)
    n_sb = 0
    for c in range(ncores):
        for h in range(2):
            lo = (2 * c + h) * ch
            src, seg_of_w = _pad_runs(xb[lo:lo + ch], lo, ch)
            halves.append((src, seg_of_w))
            n_sb = max(n_sb, len(seg_of_w))

    cols = n_sb * SB
    nwin = 2 * n_sb                  # P columns per core
    nch = -(-nwin // 128)            # 128-row chunks of the routing matmul
    nwp = 128 * nch

    xts = []
    winseg = np.full((ncores, nwp), -1, dtype=np.int64)   # window -> segment
    winpad = np.full((ncores, nwp), SB, dtype=np.float64)  # pad cols per window
    for c in range(ncores):
        xt = np.zeros((128, cols), dtype=F8NP)
        for h in range(2):
            src, seg_of_w = halves[2 * c + h]
            if len(src) < cols:
                src = np.concatenate([src, np.full(cols - len(src), -1, np.int64)])
            mask = src >= 0
            gath = np.zeros((cols, 64), dtype=np.float32)
            gath[mask] = x[src[mask]]
            xt[64 * h:64 * h + 64, :] = gath.T.astype(F8NP)
            pads = SB - mask.reshape(-1, SB).sum(axis=1)   # per window of half
            for i, s in enumerate(seg_of_w):
                w = 2 * i + h                              # psa -> even, psb -> odd
                winseg[c, w] = s
                winpad[c, w] = float(pads[i])
        xts.append(xt)

    # device-owned segments: fully contained in one core's node range
    owner = np.full(NSEG, -1, dtype=np.int64)
    slots = []                                # per core: list of seg ids (<=128)
    extra = []                                # per core: owned but not slotted
    for c in range(ncores):
        lo, hi = 2 * c * ch, 2 * (c + 1) * ch
        segs = np.unique(xb[lo:hi])
        owned = []
        for s in segs:
            s = int(s)
            first, last = np.searchsorted(xb, [s, s + 1])
            if first >= lo and last <= hi:
                owned.append(s)
        slots.append(owned[:NSLOT])
        extra.append(owned[NSLOT:])
        for s in owned[:NSLOT]:
            owner[s] = c

    # routing matrices R [nwp, NSLOT] with 1/count folded in
    Rs = []
    for c in range(ncores):
        R = np.zeros((nwp, NSLOT), dtype=np.float32)
        slot_of = {s: i for i, s in enumerate(slots[c])}
        for w in range(nwin):
            s = winseg[c, w]
            if s >= 0 and s in slot_of:
                R[w, slot_of[s]] = 1.0 / max(counts[s], 1.0)
        Rs.append(R)

    meta = dict(n_sb=n_sb, cols=cols, nwin=nwin, nch=nch, nwp=nwp,
                counts=counts, winseg=winseg, winpad=winpad,
                slots=slots, extra=extra, owner=owner, Rs=Rs, ncores=ncores)
    return xts, meta


def _build(n_sb, nch, ncores=NCORES):
    nc = bacc.Bacc("TRN2", target_bir_lowering=False, debug=False,
                   num_devices=ncores)
    cols = n_sb * SB
    nwp = 128 * nch
    xt_d = nc.dram_tensor("xt", [128, cols], FP8, kind="ExternalInput").ap()
    wza_d = nc.dram_tensor("wza", [128, 128], FP8, kind="ExternalInput").ap()
    wzb_d = nc.dram_tensor("wzb", [128, 128], FP8, kind="ExternalInput").ap()
    b1_d = nc.dram_tensor("b1", [128, 1], F32, kind="ExternalInput").ap()
    nb1_d = nc.dram_tensor("nb1", [128, 1], F32, kind="ExternalInput").ap()
    corw_d = nc.dram_tensor("corw", [32, 128], F32, kind="ExternalInput").ap()
    corv_d = nc.dram_tensor("corv", [32, nwp], F32, kind="ExternalInput").ap()
    g_d = nc.dram_tensor("g", [128, 128], F32, kind="ExternalInput").ap()
    r_d = nc.dram_tensor("r", [128, nch * NSLOT], F32, kind="ExternalInput").ap()
    ident_d = nc.dram_tensor("ident", [128, 128], F32, kind="ExternalInput").ap()
    c3_d = nc.dram_tensor("c3", [128, 1], F32, kind="ExternalInput").ap()
    w4_d = nc.dram_tensor("w4", [128, ODIM], BF16, kind="ExternalInput").ap()
    b4_d = nc.dram_tensor("b4", [ODIM, 1], F32, kind="ExternalInput").ap()
    p_d = nc.dram_tensor("p_out", [128, nwp], F32, kind="ExternalOutput").ap()
    out_d = nc.dram_tensor("out_t", [ODIM, NSLOT], F32, kind="ExternalOutput").ap()

    CH = SB_PER_CHUNK * SB

    with tile.TileContext(nc) as tc:
        with tc.tile_pool(name="const", bufs=1) as cpool, \
             tc.tile_pool(name="xin", bufs=3) as xpool, \
             tc.tile_pool(name="tr", bufs=1) as trpool, \
             tc.tile_pool(name="ps", bufs=2, space="PSUM") as pspool:

            wza = cpool.tile([128, 128], FP8)
            nc.sync.dma_start(wza[:], wza_d[:])
            wzb = cpool.tile([128, 128], FP8)
            nc.sync.dma_start(wzb[:], wzb_d[:])
            b1t = cpool.tile([128, 1], F32)
            nc.sync.dma_start(b1t[:], b1_d[:])
            nb1t = cpool.tile([128, 1], F32)
            nc.sync.dma_start(nb1t[:], nb1_d[:])
            corw = cpool.tile([32, 128], F32)
            nc.sync.dma_start(corw[:], corw_d[:])
            corv = cpool.tile([32, nwp], F32)
            nc.sync.dma_start(corv[:], corv_d[:])
            gt = cpool.tile([128, 128], F32)
            nc.sync.dma_start(gt[:], g_d[:])
            rt = cpool.tile([128, nch * NSLOT], F32)
            nc.sync.dma_start(rt[:], r_d[:])
            ident = cpool.tile([128, 128], F32)
            nc.sync.dma_start(ident[:], ident_d[:])
            c3t = cpool.tile([128, 1], F32)
            nc.sync.dma_start(c3t[:], c3_d[:])
            w4t = cpool.tile([128, ODIM], BF16)
            nc.sync.dma_start(w4t[:], w4_d[:])
            b4t = cpool.tile([ODIM, 1], F32)
            nc.sync.dma_start(b4t[:], b4_d[:])

            P = cpool.tile([128, nwp], F32)
            nc.vector.memset(P[:], 0.0)
            trash_a = trpool.tile([128, SB], BF16, tag="ta")
            trash_d = trpool.tile([128, SB], BF16, tag="td")

            # ---- main loop: phi matmuls + fused relu/window-sum ----
            xtile = None
            for sb in range(n_sb):
                if sb % SB_PER_CHUNK == 0:
                    xtile = xpool.tile([128, CH], FP8, tag="x")
                    span = min(CH, cols - sb * SB)
                    nc.sync.dma_start(xtile[:, 0:span],
                                      xt_d[:, sb * SB:sb * SB + span])
                base = (sb % SB_PER_CHUNK) * SB
                psa = pspool.tile([128, SB], F32, tag="psa")
                psb = pspool.tile([128, SB], F32, tag="psb")
                for t in range(SB // 512):
                    off = base + 512 * t
                    nc.tensor.matmul(
                        psa[:, 512 * t:512 * t + 512], lhsT=wza[:],
                        rhs=xtile[:, off:off + 512], start=True, stop=True)
                for t in range(SB // 512):
                    off = base + 512 * t
                    nc.tensor.matmul(
                        psb[:, 512 * t:512 * t + 512], lhsT=wzb[:],
                        rhs=xtile[:, off:off + 512], start=True, stop=True)
                # half A window -> ACT (exact relu(z+b1) sum)
                nc.scalar.activation(
                    out=trash_a[:], in_=psa[:],
                    func=mybir.ActivationFunctionType.Relu,
                    bias=b1t[:, 0:1],
                    accum_out=P[:, 2 * sb:2 * sb + 1])
                # half B window -> DVE: sum of max(z,-b1) = relu sum with
                # -b1-per-real-col offset, fixed by the correction matmul
                nc.vector.tensor_scalar(
                    out=trash_d[:], in0=psb[:],
                    scalar1=nb1t[:, 0:1], scalar2=0.0,
                    op0=mybir.AluOpType.max, op1=mybir.AluOpType.add,
                    accum_out=P[:, 2 * sb + 1:2 * sb + 2])

            # ---- correction + rho tail ----
            Pc = cpool.tile([128, nwp], F32)
            H = cpool.tile([128, nch * 128], F32)
            M2 = cpool.tile([128, 128], F32)
            T3 = cpool.tile([128, 128], BF16)
            OT = cpool.tile([ODIM, NSLOT], F32)

            psc = pspool.tile([128, SB], F32, tag="psa")
            for j in range(nch):
                sl = slice(128 * j, 128 * j + 128)
                nc.tensor.matmul(psc[:, 0:128], lhsT=corw[:], rhs=corv[:, sl],
                                 start=True, stop=True)
                nc.vector.tensor_tensor(out=Pc[:, sl], in0=P[:, sl],
                                        in1=psc[:, 0:128],
                                        op=mybir.AluOpType.add)
            nc.sync.dma_start(p_d[:], Pc[:])

            ps1 = pspool.tile([128, SB], F32, tag="psb")
            for k in range(nch):
                sl = slice(128 * k, 128 * k + 128)
                nc.tensor.matmul(ps1[:, 0:128], lhsT=Pc[:, sl], rhs=gt[:],
                                 start=True, stop=True)
                nc.vector.tensor_copy(H[:, sl], ps1[:, 0:128])

            ps2 = pspool.tile([128, SB], F32, tag="psa")
            for k in range(nch):
                sl = slice(128 * k, 128 * k + 128)
                nc.tensor.matmul(ps2[:, 0:128],
                                 lhsT=rt[:, NSLOT * k:NSLOT * k + NSLOT],
                                 rhs=H[:, sl],
                                 start=(k == 0), stop=(k == nch - 1))
            nc.vector.tensor_copy(M2[:], ps2[:, 0:128])

            ps3 = pspool.tile([128, SB], F32, tag="psb")
            nc.tensor.transpose(ps3[:, 0:128], M2[:], ident[:])
            nc.scalar.activation(
                out=T3[:], in_=ps3[:, 0:128],
                func=mybir.ActivationFunctionType.Relu,
                bias=c3t[:, 0:1])

            ps4 = pspool.tile([128, SB], F32, tag="psa")
            nc.tensor.matmul(ps4[0:ODIM, 0:128], lhsT=w4t[:], rhs=T3[:],
                             start=True, stop=True)
            nc.vector.tensor_scalar(
                out=OT[:], in0=ps4[0:ODIM, 0:128],
                scalar1=b4t[:, 0:1], scalar2=None,
                op0=mybir.AluOpType.add)
            nc.sync.dma_start(out_d[:], OT[:])

    nc.compile()
    return nc


def run(inputs, ncores=NCORES, trace=False):
    x = np.asarray(inputs["x"], dtype=np.float32)
    xb = np.asarray(inputs["x_batch"])
    W1 = np.asarray(inputs["W1"], dtype=np.float32)
    b1 = np.asarray(inputs["b1"], dtype=np.float32)
    W2 = np.asarray(inputs["W2"], dtype=np.float64)
    b2 = np.asarray(inputs["b2"], dtype=np.float64)
    W3 = np.asarray(inputs["W3"], dtype=np.float64)
    b3 = np.asarray(inputs["b3"], dtype=np.float64)
    W4 = np.asarray(inputs["W4"], dtype=np.float64)
    b4 = np.asarray(inputs["b4"], dtype=np.float64)

    xts, meta = _host_prep(x, xb, ncores=ncores)
    n_sb, nch, nwp = meta["n_sb"], meta["nch"], meta["nwp"]
    counts = meta["counts"]

    wza = np.zeros((128, 128), dtype=np.float32)
    wza[0:64, :] = W1
    wzb = np.zeros((128, 128), dtype=np.float32)
    wzb[64:128, :] = W1
    b1c = np.ascontiguousarray(b1, np.float32).reshape(128, 1)
    nb1c = np.ascontiguousarray(-b1, np.float32).reshape(128, 1)

    # correction: Pc = P + corw.T @ corv
    #   ACT window w: sum = true + padcnt_w * relu(b1)
    #   DVE window w: sum = true - realcnt_w * b1 + padcnt_w * relu(-b1)
    corw = np.zeros((32, 128), dtype=np.float32)
    corw[0, :] = b1
    corw[1, :] = np.maximum(-b1, 0.0)
    corw[2, :] = np.maximum(b1, 0.0)

    G = np.ascontiguousarray((W2 @ W3).astype(np.float32))
    c3 = np.ascontiguousarray((b2 @ W3 + b3).astype(np.float32)).reshape(128, 1)
    w4c = np.ascontiguousarray(W4.astype(np.float32)).astype(ml_dtypes.bfloat16)
    b4c = np.ascontiguousarray(b4.astype(np.float32)).reshape(ODIM, 1)
    ident = np.eye(128, dtype=np.float32)

    in_maps = []
    for c in range(ncores):
        corv = np.zeros((32, nwp), dtype=np.float32)
        isdve = (np.arange(nwp) % 2) == 1
        pad = meta["winpad"][c]
        real = SB - pad
        corv[0, :] = np.where(isdve, real, 0.0)
        corv[1, :] = np.where(isdve, -pad, 0.0)
        corv[2, :] = np.where(isdve, 0.0, -pad)
        # R stored as [128, nch*NSLOT]: chunk k rows -> cols [k*NSLOT, ...)
        R = meta["Rs"][c]
        rstack = np.zeros((128, nch * NSLOT), dtype=np.float32)
        for k in range(nch):
            rstack[:, NSLOT * k:NSLOT * (k + 1)] = R[128 * k:128 * k + 128, :]
        in_maps.append(dict(
            xt=xts[c], wza=wza.astype(F8NP), wzb=wzb.astype(F8NP),
            b1=b1c, nb1=nb1c, corw=corw, corv=corv, g=G, r=rstack,
            ident=ident, c3=c3, w4=w4c, b4=b4c))

    nc = _build(n_sb, nch, ncores=ncores)
    res = run_bass_kernel_spmd(nc, in_maps, core_ids=list(range(ncores)),
                               trace=trace)

    # ---- host assembly ----
    out = np.zeros((NSEG, ODIM), dtype=np.float32)
    done = np.zeros(NSEG, dtype=bool)
    for c in range(ncores):
        ot = res.results[c]["out_t"]          # [ODIM, NSLOT]
        for i, s in enumerate(meta["slots"][c]):
            out[s] = ot[:, i]
            done[s] = True

    # remaining segments from exported corrected partials
    rest = np.where(~done)[0]
    if len(rest):
        sums = np.zeros((NSEG, 128), dtype=np.float64)
        for c in range(ncores):
            Pc = res.results[c]["p_out"].astype(np.float64)   # [128, nwp]
            ws = meta["winseg"][c]
            valid = ws >= 0
            np.add.at(sums, ws[valid], Pc.T[valid])
        need = rest[counts[rest] > 0]
        if len(need):
            mean = sums[need] / counts[need][:, None]
            hid = mean @ W2 + b2
            t3 = np.maximum(hid @ W3 + b3, 0.0)
            out[need] = (t3 @ W4 + b4).astype(np.float32)
        empty = rest[counts[rest] == 0]
        if len(empty):
            row = (np.maximum(b3, 0.0) @ W4 + b4).astype(np.float32)
            out[empty] = row
    return out, res, None


def kernel(**inputs):
    inputs = {k: np.asarray(v) for k, v in inputs.items()}
    out, _, _ = run(inputs)
    return out


if __name__ == "__main__":
    rng = np.random.default_rng(0)
    N, D, HN, B = 8 * 24 * SB, 64, 128, 256
    x = rng.standard_normal((N, D), dtype=np.float32)
    xb = np.sort(rng.integers(0, B, N).astype(np.int32))
    W1 = (rng.standard_normal((D, HN)) / 8).astype(np.float32)
    W2 = (rng.standard_normal((HN, HN)) / 11.3).astype(np.float32)
    W3 = (rng.standard_normal((HN, HN)) / 11.3).astype(np.float32)
    W4 = (rng.standard_normal((HN, ODIM)) / 11.3).astype(np.float32)
    b1 = rng.standard_normal(HN).astype(np.float32) * 0.1
    b2 = rng.standard_normal(HN).astype(np.float32) * 0.1
    b3 = rng.standard_normal(HN).astype(np.float32) * 0.1
    b4 = rng.standard_normal(ODIM).astype(np.float32) * 0.1
    ins = dict(x=x, x_batch=xb, W1=W1, b1=b1, W2=W2, b2=b2, W3=W3, b3=b3,
               W4=W4, b4=b4)
    out = kernel(**ins)

    h = np.maximum(x @ W1 + b1, 0) @ W2 + b2
    sums = np.zeros((1024, HN), dtype=np.float64)
    np.add.at(sums, xb, h.astype(np.float64))
    cnt = np.bincount(xb, minlength=1024).astype(np.float64)
    mean = sums / np.maximum(cnt, 1)[:, None]
    ref = (np.maximum(mean @ W3 + b3, 0) @ W4 + b4).astype(np.float32)
    num = np.linalg.norm(out - ref)
    den = np.linalg.norm(ref)
    print("Relative error:", num / den)


# revision 5
# speedup vs baseline: 1.2647x; 1.0286x over previous
"""DeepSet segment-reduce kernel for 8 Trainium2 NeuronCores.

Single fused NEFF per core (SPMD over 8 cores):
  1. phi matmul: z = x @ W1 on the PE (fp8 inputs, K=128 with the two
     node-halves packed on SBUF partition halves and zero-padded fp8
     weights wza=[[W1],[0]], wzb=[[0],[W1]]).
  2. fused relu+window-sum: per 1024-column window, one ACT activation
     (half A, written back in place to PSUM) or DVE tensor_scalar
     (half B) with accum_out writes the [128,1] window partial into P.
     Windows are segment-aligned via per-run padding (~5% pad), so
     each window belongs to one segment.
  3. bias/pad correction per 128-window chunk (overlapped with the
     stream): a K=32 matmul builds the exact correction (pad columns
     contribute relu(b1) on the ACT path and relu(-b1) on the DVE
     path, real DVE columns are offset by -b1), added to P -> Pc, and
     H_k = Pc_k.T @ G is evicted (G = W2@W3, c3 = b2@W3+b3 folded).
  4. rho tail on device: routing matmuls (lhsT = R chunk with the
     window->segment map and 1/count folded in) accumulate mean@G in
     PSUM, a PE transpose restores [hid, seg] orientation, ACT applies
     relu(. + c3), and a final W4 matmul + b4 bias produces out rows
     for 128 device-owned segments per core.
  5. host: segments not device-owned (core boundaries, leftovers,
     empties) are recomputed exactly from the exported Pc partials.
"""

import os
import sys

for _p in ("/opt/trn_rl_repo",):
    if os.path.isdir(_p) and _p not in sys.path:
        sys.path.append(_p)

import numpy as np
import ml_dtypes

import concourse.bass as bass
import concourse.tile as tile
from concourse import bacc, mybir
from concourse.bass_utils import run_bass_kernel_spmd

F32 = mybir.dt.float32
BF16 = mybir.dt.bfloat16
FP8 = mybir.dt.float8e4
F8NP = ml_dtypes.float8_e4m3

NCORES = 8
SB = 1024            # window columns (one [128,1024] fp32 psum tile)
NSEG = 1024
ODIM = 16
NSLOT = 128          # device-owned segments per core
WPC = 128            # P columns per chunk (= 64 superblocks)


def _chunk_sizes(n_sb):
    """DMA chunk schedule in superblocks: small first chunk so the PE
    starts early, then 8-superblock chunks."""
    sizes = [min(2, n_sb)]
    left = n_sb - sizes[0]
    while left > 0:
        sizes.append(min(8, left))
        left -= sizes[-1]
    return sizes


def _pad_runs(ids, lo, ch):
    """Segment runs of a sorted id slice, padded to SB multiples.
    Returns (src indices with -1 pads, seg id per window)."""
    uniq, starts = np.unique(ids, return_index=True)
    ends = np.append(starts[1:], ch)
    seg_of_w = []
    src_parts = []
    for k in range(len(uniq)):
        L = int(ends[k] - starts[k])
        T = -(-L // SB)
        arr = np.full(T * SB, -1, dtype=np.int64)
        arr[:L] = lo + starts[k] + np.arange(L)
        src_parts.append(arr)
        seg_of_w += [int(uniq[k])] * T
    src = np.concatenate(src_parts) if src_parts else np.empty(0, np.int64)
    return src, seg_of_w


def _host_prep(x, x_batch, ncores=NCORES):
    N = x.shape[0]
    assert N % (2 * ncores) == 0
    ch = N // (2 * ncores)
    xb = np.asarray(x_batch)

    counts = np.bincount(xb, minlength=NSEG).astype(np.float64)

    # half-split at the segment boundary nearest each core's midpoint
    # (avoids splitting the middle segment into two padded runs)
    halves = []                      # (src, seg_of_w, lo, size) per (core, half)
    n_sb = 0
    for c in range(ncores):
        lo0, hi0 = 2 * c * ch, 2 * (c + 1) * ch
        mid = lo0 + ch
        s_mid = int(xb[min(mid, N - 1)])
        first, last = np.searchsorted(xb, [s_mid, s_mid + 1])
        cand = [b for b in (first, last) if lo0 < b < hi0]
        split = min(cand, key=lambda b: abs(b - mid)) if cand else mid
        for (a, b) in ((lo0, split), (split, hi0)):
            src, seg_of_w = _pad_runs(xb[a:b], a, b - a)
            halves.append((src, seg_of_w))
            n_sb = max(n_sb, len(seg_of_w))

    cols = n_sb * SB
    nwin = 2 * n_sb
    nch = -(-nwin // WPC)
    nwp = WPC * nch

    xts = []
    winseg = np.full((ncores, nwp), -1, dtype=np.int64)
    winpad = np.full((ncores, nwp), SB, dtype=np.float64)
    for c in range(ncores):
        xt = np.zeros((128, cols), dtype=F8NP)
        for h in range(2):
            src, seg_of_w = halves[2 * c + h]
            if len(src) < cols:
                src = np.concatenate([src, np.full(cols - len(src), -1, np.int64)])
            mask = src >= 0
            gath = np.zeros((cols, 64), dtype=np.float32)
            gath[mask] = x[src[mask]]
            xt[64 * h:64 * h + 64, :] = gath.T.astype(F8NP)
            pads = SB - mask.reshape(-1, SB).sum(axis=1)
            for i, s in enumerate(seg_of_w):
                w = 2 * i + h
                winseg[c, w] = s
                winpad[c, w] = float(pads[i])
        xts.append(xt)

    # device-owned segments: fully contained in one core's node range
    slots = []
    for c in range(ncores):
        lo, hi = 2 * c * ch, 2 * (c + 1) * ch
        segs = np.unique(xb[lo:hi])
        owned = []
        for s in segs:
            s = int(s)
            first, last = np.searchsorted(xb, [s, s + 1])
            if first >= lo and last <= hi:
                owned.append(s)
        slots.append(owned[:NSLOT])

    # routing matrices R [nwp, NSLOT] with 1/count folded in
    Rs = []
    for c in range(ncores):
        R = np.zeros((nwp, NSLOT), dtype=np.float32)
        slot_of = {s: i for i, s in enumerate(slots[c])}
        for w in range(nwin):
            s = winseg[c, w]
            if s >= 0 and s in slot_of:
                R[w, slot_of[s]] = 1.0 / max(counts[s], 1.0)
        Rs.append(R)

    meta = dict(n_sb=n_sb, cols=cols, nwin=nwin, nch=nch, nwp=nwp,
                counts=counts, winseg=winseg, winpad=winpad,
                slots=slots, Rs=Rs, ncores=ncores)
    return xts, meta


def _build(n_sb, nch, ncores=NCORES):
    nc = bacc.Bacc("TRN2", target_bir_lowering=False, debug=False,
                   num_devices=ncores)
    cols = n_sb * SB
    nwp = WPC * nch
    xt_d = nc.dram_tensor("xt", [128, cols], FP8, kind="ExternalInput").ap()
    wza_d = nc.dram_tensor("wza", [128, 128], FP8, kind="ExternalInput").ap()
    wzb_d = nc.dram_tensor("wzb", [128, 128], FP8, kind="ExternalInput").ap()
    b1_d = nc.dram_tensor("b1", [128, 1], F32, kind="ExternalInput").ap()
    nb1_d = nc.dram_tensor("nb1", [128, 1], F32, kind="ExternalInput").ap()
    corw_d = nc.dram_tensor("corw", [32, 128], F32, kind="ExternalInput").ap()
    corv_d = nc.dram_tensor("corv", [32, nwp], F32, kind="ExternalInput").ap()
    g_d = nc.dram_tensor("g", [128, 128], F32, kind="ExternalInput").ap()
    r_d = nc.dram_tensor("r", [128, nch * NSLOT], F32, kind="ExternalInput").ap()
    ident_d = nc.dram_tensor("ident", [128, 128], F32, kind="ExternalInput").ap()
    c3_d = nc.dram_tensor("c3", [128, 1], F32, kind="ExternalInput").ap()
    w4_d = nc.dram_tensor("w4", [128, ODIM], BF16, kind="ExternalInput").ap()
    b4_d = nc.dram_tensor("b4", [ODIM, 1], F32, kind="ExternalInput").ap()
    p_d = nc.dram_tensor("p_out", [128, nwp], F32, kind="ExternalOutput").ap()
    out_d = nc.dram_tensor("out_t", [ODIM, NSLOT], F32, kind="ExternalOutput").ap()

    sizes = _chunk_sizes(n_sb)
    starts = np.cumsum([0] + sizes[:-1])

    with tile.TileContext(nc) as tc:
        with tc.tile_pool(name="const", bufs=1) as cpool, \
             tc.tile_pool(name="xin", bufs=3) as xpool, \
             tc.tile_pool(name="tr", bufs=1) as trpool, \
             tc.tile_pool(name="ps", bufs=2, space="PSUM") as pspool:

            # first x chunk before the constants: PE starts ~1us in
            xtile = xpool.tile([128, sizes[0] * SB], FP8, tag="x")
            nc.sync.dma_start(xtile[:], xt_d[:, 0:sizes[0] * SB])

            wza = cpool.tile([128, 128], FP8)
            nc.sync.dma_start(wza[:], wza_d[:])
            wzb = cpool.tile([128, 128], FP8)
            nc.sync.dma_start(wzb[:], wzb_d[:])
            b1t = cpool.tile([128, 1], F32)
            nc.sync.dma_start(b1t[:], b1_d[:])
            nb1t = cpool.tile([128, 1], F32)
            nc.sync.dma_start(nb1t[:], nb1_d[:])
            corw = cpool.tile([32, 128], F32)
            nc.sync.dma_start(corw[:], corw_d[:])
            corv = cpool.tile([32, nwp], F32)
            nc.sync.dma_start(corv[:], corv_d[:])
            gt = cpool.tile([128, 128], F32)
            nc.sync.dma_start(gt[:], g_d[:])
            rt = cpool.tile([128, nch * NSLOT], F32)
            nc.sync.dma_start(rt[:], r_d[:])
            ident = cpool.tile([128, 128], F32)
            nc.sync.dma_start(ident[:], ident_d[:])
            c3t = cpool.tile([128, 1], F32)
            nc.sync.dma_start(c3t[:], c3_d[:])
            w4t = cpool.tile([128, ODIM], BF16)
            nc.sync.dma_start(w4t[:], w4_d[:])
            b4t = cpool.tile([ODIM, 1], F32)
            nc.sync.dma_start(b4t[:], b4_d[:])

            P = [cpool.tile([128, WPC], F32, name=f"P{k}") for k in range(nch)]
            for Pk in P:
                nc.vector.memset(Pk[:], 0.0)
            Pc = [cpool.tile([128, WPC], F32, name=f"Pc{k}") for k in range(nch)]
            H = [cpool.tile([128, 128], F32, name=f"H{k}") for k in range(nch)]
            trash_d = trpool.tile([128, SB], BF16, tag="td")

            def chunk_tail(k):
                """correction + Pc + H_k for chunk k; overlapped with
                the stream (only transient psum tiles)."""
                sl = slice(WPC * k, WPC * (k + 1))
                psc = pspool.tile([128, SB], F32, tag="psa")
                nc.tensor.matmul(psc[:, 0:WPC], lhsT=corw[:], rhs=corv[:, sl],
                                 start=True, stop=True)
                nc.vector.tensor_tensor(out=Pc[k][:], in0=P[k][:],
                                        in1=psc[:, 0:WPC],
                                        op=mybir.AluOpType.add)
                nc.sync.dma_start(p_d[:, sl], Pc[k][:])
                ps1 = pspool.tile([128, SB], F32, tag="psb")
                nc.tensor.matmul(ps1[:, 0:128], lhsT=Pc[k][:], rhs=gt[:],
                                 start=True, stop=True)
                nc.scalar.copy(out=H[k][:], in_=ps1[:, 0:128])

            done_chunks = 0
            ci = 0
            for sb in range(n_sb):
                if ci < len(sizes) and sb == starts[ci]:
                    if ci > 0:
                        xtile = xpool.tile([128, sizes[ci] * SB], FP8, tag="x")
                        lo = starts[ci] * SB
                        nc.sync.dma_start(xtile[:],
                                          xt_d[:, lo:lo + sizes[ci] * SB])
                    ci += 1
                base = (sb - starts[ci - 1]) * SB
                psa = pspool.tile([128, SB], F32, tag="psa")
                psb = pspool.tile([128, SB], F32, tag="psb")
                for t in range(SB // 512):
                    off = base + 512 * t
                    nc.tensor.matmul(
                        psa[:, 512 * t:512 * t + 512], lhsT=wza[:],
                        rhs=xtile[:, off:off + 512], start=True, stop=True)
                for t in range(SB // 512):
                    off = base + 512 * t
                    nc.tensor.matmul(
                        psb[:, 512 * t:512 * t + 512], lhsT=wzb[:],
                        rhs=xtile[:, off:off + 512], start=True, stop=True)
                wa, wb = 2 * sb, 2 * sb + 1
                # half A -> ACT, relu written back in place to PSUM
                nc.scalar.activation(
                    out=psa[:], in_=psa[:],
                    func=mybir.ActivationFunctionType.Relu,
                    bias=b1t[:, 0:1],
                    accum_out=P[wa // WPC][:, wa % WPC:wa % WPC + 1])
                # half B -> DVE
                nc.vector.tensor_scalar(
                    out=trash_d[:], in0=psb[:],
                    scalar1=nb1t[:, 0:1], scalar2=0.0,
                    op0=mybir.AluOpType.max, op1=mybir.AluOpType.add,
                    accum_out=P[wb // WPC][:, wb % WPC:wb % WPC + 1])
                while (done_chunks + 1) * WPC <= 2 * (sb + 1):
                    chunk_tail(done_chunks)
                    done_chunks += 1
            while done_chunks < nch:
                chunk_tail(done_chunks)
                done_chunks += 1

            # ---- rho tail ----
            M2 = cpool.tile([128, 128], F32)
            T3 = cpool.tile([128, 128], BF16)
            OT = cpool.tile([ODIM, NSLOT], F32)

            ps2 = pspool.tile([128, SB], F32, tag="psa")
            for k in range(nch):
                nc.tensor.matmul(ps2[:, 0:128],
                                 lhsT=rt[:, NSLOT * k:NSLOT * k + NSLOT],
                                 rhs=H[k][:],
                                 start=(k == 0), stop=(k == nch - 1))
            nc.vector.tensor_copy(M2[:], ps2[:, 0:128])

            ps3 = pspool.tile([128, SB], F32, tag="psb")
            nc.tensor.transpose(ps3[:, 0:128], M2[:], ident[:])
            nc.scalar.activation(
                out=T3[:], in_=ps3[:, 0:128],
                func=mybir.ActivationFunctionType.Relu,
                bias=c3t[:, 0:1])

            ps4 = pspool.tile([128, SB], F32, tag="psa")
            nc.tensor.matmul(ps4[0:ODIM, 0:128], lhsT=w4t[:], rhs=T3[:],
                             start=True, stop=True)
            nc.vector.tensor_scalar(
                out=OT[:], in0=ps4[0:ODIM, 0:128],
                scalar1=b4t[:, 0:1], scalar2=None,
                op0=mybir.AluOpType.add)
            nc.sync.dma_start(out_d[:], OT[:])

    nc.compile()
    return nc


def run(inputs, ncores=NCORES, trace=False):
    x = np.asarray(inputs["x"], dtype=np.float32)
    xb = np.asarray(inputs["x_batch"])
    W1 = np.asarray(inputs["W1"], dtype=np.float32)
    b1 = np.asarray(inputs["b1"], dtype=np.float32)
    W2 = np.asarray(inputs["W2"], dtype=np.float64)
    b2 = np.asarray(inputs["b2"], dtype=np.float64)
    W3 = np.asarray(inputs["W3"], dtype=np.float64)
    b3 = np.asarray(inputs["b3"], dtype=np.float64)
    W4 = np.asarray(inputs["W4"], dtype=np.float64)
    b4 = np.asarray(inputs["b4"], dtype=np.float64)

    xts, meta = _host_prep(x, xb, ncores=ncores)
    n_sb, nch, nwp = meta["n_sb"], meta["nch"], meta["nwp"]
    counts = meta["counts"]

    wza = np.zeros((128, 128), dtype=np.float32)
    wza[0:64, :] = W1
    wzb = np.zeros((128, 128), dtype=np.float32)
    wzb[64:128, :] = W1
    b1c = np.ascontiguousarray(b1, np.float32).reshape(128, 1)
    nb1c = np.ascontiguousarray(-b1, np.float32).reshape(128, 1)

    # correction: Pc = P + corw.T @ corv
    #   ACT window w: sum = true + padcnt_w * relu(b1)
    #   DVE window w: sum = true - realcnt_w * b1 + padcnt_w * relu(-b1)
    corw = np.zeros((32, 128), dtype=np.float32)
    corw[0, :] = b1
    corw[1, :] = np.maximum(-b1, 0.0)
    corw[2, :] = np.maximum(b1, 0.0)

    G = np.ascontiguousarray((W2 @ W3).astype(np.float32))
    c3 = np.ascontiguousarray((b2 @ W3 + b3).astype(np.float32)).reshape(128, 1)
    w4c = np.ascontiguousarray(W4.astype(np.float32)).astype(ml_dtypes.bfloat16)
    b4c = np.ascontiguousarray(b4.astype(np.float32)).reshape(ODIM, 1)
    ident = np.eye(128, dtype=np.float32)

    in_maps = []
    for c in range(ncores):
        corv = np.zeros((32, nwp), dtype=np.float32)
        isdve = (np.arange(nwp) % 2) == 1
        pad = meta["winpad"][c]
        real = SB - pad
        corv[0, :] = np.where(isdve, real, 0.0)
        corv[1, :] = np.where(isdve, -pad, 0.0)
        corv[2, :] = np.where(isdve, 0.0, -pad)
        R = meta["Rs"][c]
        rstack = np.zeros((128, nch * NSLOT), dtype=np.float32)
        for k in range(nch):
            rstack[:, NSLOT * k:NSLOT * (k + 1)] = R[WPC * k:WPC * k + WPC, :]
        in_maps.append(dict(
            xt=xts[c], wza=wza.astype(F8NP), wzb=wzb.astype(F8NP),
            b1=b1c, nb1=nb1c, corw=corw, corv=corv, g=G, r=rstack,
            ident=ident, c3=c3, w4=w4c, b4=b4c))

    nc = _build(n_sb, nch, ncores=ncores)
    res = run_bass_kernel_spmd(nc, in_maps, core_ids=list(range(ncores)),
                               trace=trace)

    # ---- host assembly ----
    out = np.zeros((NSEG, ODIM), dtype=np.float32)
    done = np.zeros(NSEG, dtype=bool)
    for c in range(ncores):
        ot = res.results[c]["out_t"]
        for i, s in enumerate(meta["slots"][c]):
            out[s] = ot[:, i]
            done[s] = True

    rest = np.where(~done)[0]
    if len(rest):
        sums = np.zeros((NSEG, 128), dtype=np.float64)
        for c in range(ncores):
            Pc = res.results[c]["p_out"].astype(np.float64)
            ws = meta["winseg"][c]
            valid = ws >= 0
            np.add.at(sums, ws[valid], Pc.T[valid])
        need = rest[counts[rest] > 0]
        if len(need):
            mean = sums[need] / counts[need][:, None]
            hid = mean @ W2 + b2
            t3 = np.maximum(hid @ W3 + b3, 0.0)
            out[need] = (t3 @ W4 + b4).astype(np.float32)
        empty = rest[counts[rest] == 0]
        if len(empty):
            row = (np.maximum(b3, 0.0) @ W4 + b4).astype(np.float32)
            out[empty] = row
    return out, res, None


def kernel(**inputs):
    inputs = {k: np.asarray(v) for k, v in inputs.items()}
    out, _, _ = run(inputs)
    return out


if __name__ == "__main__":
    rng = np.random.default_rng(0)
    N, D, HN, B = 8 * 24 * SB, 64, 128, 256
    x = rng.standard_normal((N, D), dtype=np.float32)
    xb = np.sort(rng.integers(0, B, N).astype(np.int32))
    W1 = (rng.standard_normal((D, HN)) / 8).astype(np.float32)
    W2 = (rng.standard_normal((HN, HN)) / 11.3).astype(np.float32)
    W3 = (rng.standard_normal((HN, HN)) / 11.3).astype(np.float32)
    W4 = (rng.standard_normal((HN, ODIM)) / 11.3).astype(np.float32)
    b1 = rng.standard_normal(HN).astype(np.float32) * 0.1
    b2 = rng.standard_normal(HN).astype(np.float32) * 0.1
    b3 = rng.standard_normal(HN).astype(np.float32) * 0.1
    b4 = rng.standard_normal(ODIM).astype(np.float32) * 0.1
    ins = dict(x=x, x_batch=xb, W1=W1, b1=b1, W2=W2, b2=b2, W3=W3, b3=b3,
               W4=W4, b4=b4)
    out = kernel(**ins)

    h = np.maximum(x @ W1 + b1, 0) @ W2 + b2
    sums = np.zeros((1024, HN), dtype=np.float64)
    np.add.at(sums, xb, h.astype(np.float64))
    cnt = np.bincount(xb, minlength=1024).astype(np.float64)
    mean = sums / np.maximum(cnt, 1)[:, None]
    ref = (np.maximum(mean @ W3 + b3, 0) @ W4 + b4).astype(np.float32)
    num = np.linalg.norm(out - ref)
    den = np.linalg.norm(ref)
    print("Relative error:", num / den)


# revision 8
# speedup vs baseline: 1.2834x; 1.0148x over previous
"""DeepSet segment-reduce kernel for 8 Trainium2 NeuronCores.

Single fused NEFF per core (SPMD over 8 cores):
  1. phi matmul: z = x @ W1 on the PE (fp8 inputs, K=128 with the two
     node-halves packed on SBUF partition halves and zero-padded fp8
     weights wza=[[W1],[0]], wzb=[[0],[W1]]).
  2. fused relu+window-sum: per 1024-column window, one ACT activation
     (half A, written back in place to PSUM) or DVE tensor_scalar
     (half B) with accum_out writes the [128,1] window partial into P.
     Windows are segment-aligned via per-run padding (~5% pad), so
     each window belongs to one segment.
  3. bias/pad correction per 128-window chunk (overlapped with the
     stream): a K=32 matmul builds the exact correction (pad columns
     contribute relu(b1) on the ACT path and relu(-b1) on the DVE
     path, real DVE columns are offset by -b1), added to P -> Pc, and
     H_k = Pc_k.T @ G is evicted (G = W2@W3, c3 = b2@W3+b3 folded).
  4. rho tail on device: routing matmuls (lhsT = R chunk with the
     window->segment map and 1/count folded in) accumulate mean@G in
     PSUM, a PE transpose restores [hid, seg] orientation, ACT applies
     relu(. + c3), and a final W4 matmul + b4 bias produces out rows
     for 128 device-owned segments per core.
  5. host: segments not device-owned (core boundaries, leftovers,
     empties) are recomputed exactly from the exported Pc partials.
"""

import os
import sys

for _p in ("/opt/trn_rl_repo",):
    if os.path.isdir(_p) and _p not in sys.path:
        sys.path.append(_p)

import numpy as np
import ml_dtypes

import concourse.bass as bass
import concourse.tile as tile
from concourse import bacc, mybir
from concourse.bass_utils import run_bass_kernel_spmd

F32 = mybir.dt.float32
BF16 = mybir.dt.bfloat16
FP8 = mybir.dt.float8e4
F8NP = ml_dtypes.float8_e4m3

NCORES = 8
SB = 1024            # window columns (one [128,1024] fp32 psum tile)
NSEG = 1024
ODIM = 16
NSLOT = 128          # device-owned segments per core
WPC = 128            # P columns per chunk (= 64 superblocks)


def _chunk_sizes(n_sb):
    """DMA chunk schedule in superblocks: ramp small so the PE starts
    early and DMA stays ahead, then 8-superblock chunks."""
    ramp = [1, 1, 2, 2, 4, 6]
    sizes = []
    left = n_sb
    for r in ramp:
        if left <= 0:
            break
        sizes.append(min(r, left))
        left -= sizes[-1]
    while left > 0:
        sizes.append(min(8, left))
        left -= sizes[-1]
    return sizes


def _pad_runs(ids, lo, ch):
    """Segment runs of a sorted id slice, padded to SB multiples.
    Returns (src indices with -1 pads, seg id per window)."""
    uniq, starts = np.unique(ids, return_index=True)
    ends = np.append(starts[1:], ch)
    seg_of_w = []
    src_parts = []
    for k in range(len(uniq)):
        L = int(ends[k] - starts[k])
        T = -(-L // SB)
        arr = np.full(T * SB, -1, dtype=np.int64)
        arr[:L] = lo + starts[k] + np.arange(L)
        src_parts.append(arr)
        seg_of_w += [int(uniq[k])] * T
    src = np.concatenate(src_parts) if src_parts else np.empty(0, np.int64)
    return src, seg_of_w


def _host_prep(x, x_batch, ncores=NCORES):
    N = x.shape[0]
    assert N % (2 * ncores) == 0
    ch = N // (2 * ncores)
    xb = np.asarray(x_batch)

    counts = np.bincount(xb, minlength=NSEG).astype(np.float64)

    # half-split at the segment boundary nearest each core's midpoint
    # (avoids splitting the middle segment into two padded runs)
    halves = []                      # (src, seg_of_w, lo, size) per (core, half)
    n_sb = 0
    for c in range(ncores):
        lo0, hi0 = 2 * c * ch, 2 * (c + 1) * ch
        mid = lo0 + ch
        s_mid = int(xb[min(mid, N - 1)])
        first, last = np.searchsorted(xb, [s_mid, s_mid + 1])
        cand = [b for b in (first, last) if lo0 < b < hi0]
        split = min(cand, key=lambda b: abs(b - mid)) if cand else mid
        for (a, b) in ((lo0, split), (split, hi0)):
            src, seg_of_w = _pad_runs(xb[a:b], a, b - a)
            halves.append((src, seg_of_w))
            n_sb = max(n_sb, len(seg_of_w))

    cols = n_sb * SB
    nwin = 2 * n_sb
    nch = -(-nwin // WPC)
    nwp = WPC * nch

    xts = []
    winseg = np.full((ncores, nwp), -1, dtype=np.int64)
    winpad = np.full((ncores, nwp), 0.0, dtype=np.float64)
    winreal = np.zeros((ncores, nwp), dtype=np.float64)
    for c in range(ncores):
        xt = np.zeros((128, cols), dtype=F8NP)
        for h in range(2):
            src, seg_of_w = halves[2 * c + h]
            if len(src) < cols:
                src = np.concatenate([src, np.full(cols - len(src), -1, np.int64)])
            mask = src >= 0
            gath = np.zeros((cols, 64), dtype=np.float32)
            gath[mask] = x[src[mask]]
            xt[64 * h:64 * h + 64, :] = gath.T.astype(F8NP)
            pads = SB - mask.reshape(-1, SB).sum(axis=1)
            for i, s in enumerate(seg_of_w):
                w = 128 * (i // 64) + 64 * h + i % 64
                winseg[c, w] = s
                winpad[c, w] = float(pads[i])
                winreal[c, w] = float(SB - pads[i])
        xts.append(xt)

    # device-owned segments: fully contained in one core's node range
    slots = []
    for c in range(ncores):
        lo, hi = 2 * c * ch, 2 * (c + 1) * ch
        segs = np.unique(xb[lo:hi])
        owned = []
        for s in segs:
            s = int(s)
            first, last = np.searchsorted(xb, [s, s + 1])
            if first >= lo and last <= hi:
                owned.append(s)
        slots.append(owned[:NSLOT])

    # routing matrices R [nwp, NSLOT] with 1/count folded in
    Rs = []
    for c in range(ncores):
        R = np.zeros((nwp, NSLOT), dtype=np.float32)
        slot_of = {s: i for i, s in enumerate(slots[c])}
        for w in range(nwp):
            s = winseg[c, w]
            if s >= 0 and s in slot_of:
                R[w, slot_of[s]] = 1.0 / max(counts[s], 1.0)
        Rs.append(R)

    meta = dict(n_sb=n_sb, cols=cols, nwin=nwin, nch=nch, nwp=nwp,
                counts=counts, winseg=winseg, winpad=winpad, winreal=winreal,
                slots=slots, Rs=Rs, ncores=ncores)
    return xts, meta


def _build(n_sb, nch, ncores=NCORES):
    nc = bacc.Bacc("TRN2", target_bir_lowering=False, debug=False,
                   num_devices=ncores)
    cols = n_sb * SB
    nwp = WPC * nch
    xt_d = nc.dram_tensor("xt", [128, cols], FP8, kind="ExternalInput").ap()
    wza_d = nc.dram_tensor("wza", [128, 128], FP8, kind="ExternalInput").ap()
    wzb_d = nc.dram_tensor("wzb", [128, 128], FP8, kind="ExternalInput").ap()
    b1_d = nc.dram_tensor("b1", [128, 1], F32, kind="ExternalInput").ap()
    nb1_d = nc.dram_tensor("nb1", [128, 1], F32, kind="ExternalInput").ap()
    corr_d = nc.dram_tensor("corr", [128, nwp], F32, kind="ExternalInput").ap()
    g_d = nc.dram_tensor("g", [128, 128], F32, kind="ExternalInput").ap()
    r_d = nc.dram_tensor("r", [128, nch * NSLOT], F32, kind="ExternalInput").ap()
    ident_d = nc.dram_tensor("ident", [128, 128], F32, kind="ExternalInput").ap()
    c3_d = nc.dram_tensor("c3", [128, 1], F32, kind="ExternalInput").ap()
    w4_d = nc.dram_tensor("w4", [128, ODIM], BF16, kind="ExternalInput").ap()
    b4_d = nc.dram_tensor("b4", [ODIM, 1], F32, kind="ExternalInput").ap()
    p_d = nc.dram_tensor("p_out", [128, nwp], F32, kind="ExternalOutput").ap()
    out_d = nc.dram_tensor("out_t", [ODIM, NSLOT], F32, kind="ExternalOutput").ap()

    sizes = _chunk_sizes(n_sb)
    starts = np.cumsum([0] + sizes[:-1])

    with tile.TileContext(nc) as tc:
        with tc.tile_pool(name="const", bufs=1) as cpool, \
             tc.tile_pool(name="xin", bufs=3) as xpool, \
             tc.tile_pool(name="tr", bufs=1) as trpool, \
             tc.tile_pool(name="ps", bufs=2, space="PSUM") as pspool:

            # first x chunk before the constants: PE starts ~1us in
            xtile = xpool.tile([128, sizes[0] * SB], FP8, tag="x")
            nc.sync.dma_start(xtile[:], xt_d[:, 0:sizes[0] * SB])

            wza = cpool.tile([128, 128], FP8)
            nc.sync.dma_start(wza[:], wza_d[:])
            wzb = cpool.tile([128, 128], FP8)
            nc.sync.dma_start(wzb[:], wzb_d[:])
            b1t = cpool.tile([128, 1], F32)
            nc.sync.dma_start(b1t[:], b1_d[:])
            nb1t = cpool.tile([128, 1], F32)
            nc.sync.dma_start(nb1t[:], nb1_d[:])
            corr = cpool.tile([128, nwp], F32)
            nc.sync.dma_start(corr[:], corr_d[:])
            gt = cpool.tile([128, 128], F32)
            nc.sync.dma_start(gt[:], g_d[:])
            rt = cpool.tile([128, nch * NSLOT], F32)
            nc.sync.dma_start(rt[:], r_d[:])
            ident = cpool.tile([128, 128], F32)
            nc.sync.dma_start(ident[:], ident_d[:])
            c3t = cpool.tile([128, 1], F32)
            nc.sync.dma_start(c3t[:], c3_d[:])
            w4t = cpool.tile([128, ODIM], BF16)
            nc.sync.dma_start(w4t[:], w4_d[:])
            b4t = cpool.tile([ODIM, 1], F32)
            nc.sync.dma_start(b4t[:], b4_d[:])

            Pa = [cpool.tile([128, 64], F32, name=f"Pa{k}") for k in range(nch)]
            Pd = [cpool.tile([128, 64], F32, name=f"Pd{k}") for k in range(nch)]
            for Pk in Pa + Pd:
                nc.vector.memset(Pk[:], 0.0)
            Pc = [cpool.tile([128, WPC], F32, name=f"Pc{k}") for k in range(nch)]
            H = [cpool.tile([128, 128], F32, name=f"H{k}") for k in range(nch)]
            trash_d = trpool.tile([128, SB], BF16, tag="td")

            def chunk_tail(k):
                """Pc = P + corr for chunk k (SBUF-only, overlapped
                with the stream) + partials export."""
                sl = slice(WPC * k, WPC * (k + 1))
                nc.vector.tensor_tensor(out=Pc[k][:, 0:64], in0=Pa[k][:],
                                        in1=corr[:, WPC * k:WPC * k + 64],
                                        op=mybir.AluOpType.add)
                nc.vector.tensor_tensor(out=Pc[k][:, 64:128], in0=Pd[k][:],
                                        in1=corr[:, WPC * k + 64:WPC * (k + 1)],
                                        op=mybir.AluOpType.add)
                nc.sync.dma_start(p_d[:, sl], Pc[k][:])

            done_chunks = 0
            ci = 0
            for sb in range(n_sb):
                if ci < len(sizes) and sb == starts[ci]:
                    if ci > 0:
                        xtile = xpool.tile([128, sizes[ci] * SB], FP8, tag="x")
                        lo = starts[ci] * SB
                        nc.sync.dma_start(xtile[:],
                                          xt_d[:, lo:lo + sizes[ci] * SB])
                    ci += 1
                base = (sb - starts[ci - 1]) * SB
                psa = pspool.tile([128, SB], F32, tag="psa")
                psb = pspool.tile([128, SB], F32, tag="psb")
                for t in range(SB // 512):
                    off = base + 512 * t
                    nc.tensor.matmul(
                        psa[:, 512 * t:512 * t + 512], lhsT=wza[:],
                        rhs=xtile[:, off:off + 512], start=True, stop=True)
                for t in range(SB // 512):
                    off = base + 512 * t
                    nc.tensor.matmul(
                        psb[:, 512 * t:512 * t + 512], lhsT=wzb[:],
                        rhs=xtile[:, off:off + 512], start=True, stop=True)
                k, j = sb // 64, sb % 64
                # half A -> ACT, relu written back in place to PSUM
                nc.scalar.activation(
                    out=psa[:], in_=psa[:],
                    func=mybir.ActivationFunctionType.Relu,
                    bias=b1t[:, 0:1],
                    accum_out=Pa[k][:, j:j + 1])
                # half B -> DVE
                nc.vector.tensor_scalar(
                    out=trash_d[:], in0=psb[:],
                    scalar1=nb1t[:, 0:1], scalar2=0.0,
                    op0=mybir.AluOpType.max, op1=mybir.AluOpType.add,
                    accum_out=Pd[k][:, j:j + 1])
                while (done_chunks + 1) * 64 <= sb + 1:
                    chunk_tail(done_chunks)
                    done_chunks += 1
            while done_chunks < nch:
                chunk_tail(done_chunks)
                done_chunks += 1

            # ---- rho tail ----
            M2 = cpool.tile([128, 128], F32)
            T3 = cpool.tile([128, 128], BF16)
            OT = cpool.tile([ODIM, NSLOT], F32)

            for k in range(nch):
                ps1 = pspool.tile([128, SB], F32, tag="psb")
                nc.tensor.matmul(ps1[:, 0:128], lhsT=Pc[k][:], rhs=gt[:],
                                 start=True, stop=True)
                nc.scalar.copy(out=H[k][:], in_=ps1[:, 0:128])

            ps2 = pspool.tile([128, SB], F32, tag="psa")
            for k in range(nch):
                nc.tensor.matmul(ps2[:, 0:128],
                                 lhsT=rt[:, NSLOT * k:NSLOT * k + NSLOT],
                                 rhs=H[k][:],
                                 start=(k == 0), stop=(k == nch - 1))
            nc.vector.tensor_copy(M2[:], ps2[:, 0:128])

            ps3 = pspool.tile([128, SB], F32, tag="psb")
            nc.tensor.transpose(ps3[:, 0:128], M2[:], ident[:])
            nc.scalar.activation(
                out=T3[:], in_=ps3[:, 0:128],
                func=mybir.ActivationFunctionType.Relu,
                bias=c3t[:, 0:1])

            ps4 = pspool.tile([128, SB], F32, tag="psa")
            nc.tensor.matmul(ps4[0:ODIM, 0:128], lhsT=w4t[:], rhs=T3[:],
                             start=True, stop=True)
            nc.vector.tensor_scalar(
                out=OT[:], in0=ps4[0:ODIM, 0:128],
                scalar1=b4t[:, 0:1], scalar2=None,
                op0=mybir.AluOpType.add)
            nc.sync.dma_start(out_d[:], OT[:])

    nc.compile()
    return nc


def run(inputs, ncores=NCORES, trace=False):
    x = np.asarray(inputs["x"], dtype=np.float32)
    xb = np.asarray(inputs["x_batch"])
    W1 = np.asarray(inputs["W1"], dtype=np.float32)
    b1 = np.asarray(inputs["b1"], dtype=np.float32)
    W2 = np.asarray(inputs["W2"], dtype=np.float64)
    b2 = np.asarray(inputs["b2"], dtype=np.float64)
    W3 = np.asarray(inputs["W3"], dtype=np.float64)
    b3 = np.asarray(inputs["b3"], dtype=np.float64)
    W4 = np.asarray(inputs["W4"], dtype=np.float64)
    b4 = np.asarray(inputs["b4"], dtype=np.float64)

    xts, meta = _host_prep(x, xb, ncores=ncores)
    n_sb, nch, nwp = meta["n_sb"], meta["nch"], meta["nwp"]
    counts = meta["counts"]

    wza = np.zeros((128, 128), dtype=np.float32)
    wza[0:64, :] = W1
    wzb = np.zeros((128, 128), dtype=np.float32)
    wzb[64:128, :] = W1
    b1c = np.ascontiguousarray(b1, np.float32).reshape(128, 1)
    nb1c = np.ascontiguousarray(-b1, np.float32).reshape(128, 1)

    G = np.ascontiguousarray((W2 @ W3).astype(np.float32))
    c3 = np.ascontiguousarray((b2 @ W3 + b3).astype(np.float32)).reshape(128, 1)
    w4c = np.ascontiguousarray(W4.astype(np.float32)).astype(ml_dtypes.bfloat16)
    b4c = np.ascontiguousarray(b4.astype(np.float32)).reshape(ODIM, 1)
    ident = np.eye(128, dtype=np.float32)

    # correction: Pc = P + corr
    #   ACT window w: sum = true + padcnt_w * relu(b1)
    #   DVE window w: sum = true - realcnt_w * b1 + padcnt_w * relu(-b1)
    in_maps = []
    for c in range(ncores):
        isdve = (np.arange(nwp) % WPC) >= 64
        pad = meta["winpad"][c]
        real = meta["winreal"][c]
        corr_t = (np.where(isdve, real, 0.0)[None, :] * b1[:, None]
                  + np.where(isdve, -pad, 0.0)[None, :]
                  * np.maximum(-b1, 0.0)[:, None]
                  + np.where(isdve, 0.0, -pad)[None, :]
                  * np.maximum(b1, 0.0)[:, None]).astype(np.float32)
        R = meta["Rs"][c]
        rstack = np.zeros((128, nch * NSLOT), dtype=np.float32)
        for k in range(nch):
            rstack[:, NSLOT * k:NSLOT * (k + 1)] = R[WPC * k:WPC * k + WPC, :]
        in_maps.append(dict(
            xt=xts[c], wza=wza.astype(F8NP), wzb=wzb.astype(F8NP),
            b1=b1c, nb1=nb1c, corr=corr_t, g=G, r=rstack,
            ident=ident, c3=c3, w4=w4c, b4=b4c))

    nc = _build(n_sb, nch, ncores=ncores)
    res = run_bass_kernel_spmd(nc, in_maps, core_ids=list(range(ncores)),
                               trace=trace)

    # ---- host assembly ----
    out = np.zeros((NSEG, ODIM), dtype=np.float32)
    done = np.zeros(NSEG, dtype=bool)
    for c in range(ncores):
        ot = res.results[c]["out_t"]
        for i, s in enumerate(meta["slots"][c]):
            out[s] = ot[:, i]
            done[s] = True

    rest = np.where(~done)[0]
    if len(rest):
        sums = np.zeros((NSEG, 128), dtype=np.float64)
        for c in range(ncores):
            Pc = res.results[c]["p_out"].astype(np.float64)
            ws = meta["winseg"][c]
            valid = ws >= 0
            np.add.at(sums, ws[valid], Pc.T[valid])
        need = rest[counts[rest] > 0]
        if len(need):
            mean = sums[need] / counts[need][:, None]
            hid = mean @ W2 + b2
            t3 = np.maximum(hid @ W3 + b3, 0.0)
            out[need] = (t3 @ W4 + b4).astype(np.float32)
        empty = rest[counts[rest] == 0]
        if len(empty):
            row = (np.maximum(b3, 0.0) @ W4 + b4).astype(np.float32)
            out[empty] = row
    return out, res, None


def kernel(**inputs):
    inputs = {k: np.asarray(v) for k, v in inputs.items()}
    out, _, _ = run(inputs)
    return out


if __name__ == "__main__":
    rng = np.random.default_rng(0)
    N, D, HN, B = 8 * 24 * SB, 64, 128, 256
    x = rng.standard_normal((N, D), dtype=np.float32)
    xb = np.sort(rng.integers(0, B, N).astype(np.int32))
    W1 = (rng.standard_normal((D, HN)) / 8).astype(np.float32)
    W2 = (rng.standard_normal((HN, HN)) / 11.3).astype(np.float32)
    W3 = (rng.standard_normal((HN, HN)) / 11.3).astype(np.float32)
    W4 = (rng.standard_normal((HN, ODIM)) / 11.3).astype(np.float32)
    b1 = rng.standard_normal(HN).astype(np.float32) * 0.1
    b2 = rng.standard_normal(HN).astype(np.float32) * 0.1
    b3 = rng.standard_normal(HN).astype(np.float32) * 0.1
    b4 = rng.standard_normal(ODIM).astype(np.float32) * 0.1
    ins = dict(x=x, x_batch=xb, W1=W1, b1=b1, W2=W2, b2=b2, W3=W3, b3=b3,
               W4=W4, b4=b4)
    out = kernel(**ins)

    h = np.maximum(x @ W1 + b1, 0) @ W2 + b2
    sums = np.zeros((1024, HN), dtype=np.float64)
    np.add.at(sums, xb, h.astype(np.float64))
    cnt = np.bincount(xb, minlength=1024).astype(np.float64)
    mean = sums / np.maximum(cnt, 1)[:, None]
    ref = (np.maximum(mean @ W3 + b3, 0) @ W4 + b4).astype(np.float32)
    num = np.linalg.norm(out - ref)
    den = np.linalg.norm(ref)
    print("Relative error:", num / den)


# revision 9
# speedup vs baseline: 1.3243x; 1.0318x over previous
"""DeepSet segment-reduce kernel for 8 Trainium2 NeuronCores.

Single fused NEFF per core (SPMD over 8 cores):
  1. phi matmul: z = x @ W1 on the PE (fp8 inputs, K=128 with the two
     node-halves packed on SBUF partition halves and zero-padded fp8
     weights wza=[[W1],[0]], wzb=[[0],[W1]]).
  2. fused relu+window-sum: per 1024-column window, one ACT activation
     (half A, written back in place to PSUM) or DVE tensor_scalar
     (half B) with accum_out writes the [128,1] window partial into P.
     Windows are segment-aligned via per-run padding (~5% pad), so
     each window belongs to one segment.
  3. bias/pad correction per 128-window chunk (overlapped with the
     stream): a K=32 matmul builds the exact correction (pad columns
     contribute relu(b1) on the ACT path and relu(-b1) on the DVE
     path, real DVE columns are offset by -b1), added to P -> Pc, and
     H_k = Pc_k.T @ G is evicted (G = W2@W3, c3 = b2@W3+b3 folded).
  4. rho tail on device: routing matmuls (lhsT = R chunk with the
     window->segment map and 1/count folded in) accumulate mean@G in
     PSUM, a PE transpose restores [hid, seg] orientation, ACT applies
     relu(. + c3), and a final W4 matmul + b4 bias produces out rows
     for 128 device-owned segments per core.
  5. host: segments not device-owned (core boundaries, leftovers,
     empties) are recomputed exactly from the exported Pc partials.
"""

import os
import sys

for _p in ("/opt/trn_rl_repo",):
    if os.path.isdir(_p) and _p not in sys.path:
        sys.path.append(_p)

import numpy as np
import ml_dtypes

import concourse.bass as bass
import concourse.tile as tile
from concourse import bacc, mybir
from concourse.bass_utils import run_bass_kernel_spmd

F32 = mybir.dt.float32
BF16 = mybir.dt.bfloat16
FP8 = mybir.dt.float8e4
F8NP = ml_dtypes.float8_e4m3

NCORES = 8
SB = 1024            # window columns (one [128,1024] fp32 psum tile)
NSEG = 1024
ODIM = 16
NSLOT = 128          # device-owned segments per core
WPC = 128            # P columns per chunk (= 64 superblocks)


def _flip(sb):
    """Superblocks whose half-B window runs on ACT instead of DVE
    (load balance: ACT is ~10% faster per window)."""
    return sb % 19 == 9


def _chunk_sizes(n_sb):
    """DMA chunk schedule in superblocks: ramp small so the PE starts
    early and DMA stays ahead, then 8-superblock chunks."""
    ramp = [1, 1, 2, 2, 4, 6]
    sizes = []
    left = n_sb
    for r in ramp:
        if left <= 0:
            break
        sizes.append(min(r, left))
        left -= sizes[-1]
    while left > 0:
        sizes.append(min(8, left))
        left -= sizes[-1]
    return sizes


def _pad_runs(ids, lo, ch):
    """Segment runs of a sorted id slice, padded to SB multiples.
    Returns (src indices with -1 pads, seg id per window)."""
    uniq, starts = np.unique(ids, return_index=True)
    ends = np.append(starts[1:], ch)
    seg_of_w = []
    src_parts = []
    for k in range(len(uniq)):
        L = int(ends[k] - starts[k])
        T = -(-L // SB)
        arr = np.full(T * SB, -1, dtype=np.int64)
        arr[:L] = lo + starts[k] + np.arange(L)
        src_parts.append(arr)
        seg_of_w += [int(uniq[k])] * T
    src = np.concatenate(src_parts) if src_parts else np.empty(0, np.int64)
    return src, seg_of_w


def _host_prep(x, x_batch, ncores=NCORES):
    N = x.shape[0]
    assert N % (2 * ncores) == 0
    ch = N // (2 * ncores)
    xb = np.asarray(x_batch)

    counts = np.bincount(xb, minlength=NSEG).astype(np.float64)

    # half-split at the segment boundary nearest each core's midpoint
    # (avoids splitting the middle segment into two padded runs)
    halves = []                      # (src, seg_of_w, lo, size) per (core, half)
    n_sb = 0
    for c in range(ncores):
        lo0, hi0 = 2 * c * ch, 2 * (c + 1) * ch
        mid = lo0 + ch
        s_mid = int(xb[min(mid, N - 1)])
        first, last = np.searchsorted(xb, [s_mid, s_mid + 1])
        cand = [b for b in (first, last) if lo0 < b < hi0]
        split = min(cand, key=lambda b: abs(b - mid)) if cand else mid
        for (a, b) in ((lo0, split), (split, hi0)):
            src, seg_of_w = _pad_runs(xb[a:b], a, b - a)
            halves.append((src, seg_of_w))
            n_sb = max(n_sb, len(seg_of_w))

    cols = n_sb * SB
    nwin = 2 * n_sb
    nch = -(-nwin // WPC)
    nwp = WPC * nch

    xts = []
    winseg = np.full((ncores, nwp), -1, dtype=np.int64)
    winpad = np.full((ncores, nwp), 0.0, dtype=np.float64)
    winreal = np.zeros((ncores, nwp), dtype=np.float64)
    for c in range(ncores):
        xt = np.zeros((128, cols), dtype=F8NP)
        for h in range(2):
            src, seg_of_w = halves[2 * c + h]
            if len(src) < cols:
                src = np.concatenate([src, np.full(cols - len(src), -1, np.int64)])
            mask = src >= 0
            gath = np.zeros((cols, 64), dtype=np.float32)
            gath[mask] = x[src[mask]]
            xt[64 * h:64 * h + 64, :] = gath.T.astype(F8NP)
            pads = SB - mask.reshape(-1, SB).sum(axis=1)
            for i, s in enumerate(seg_of_w):
                w = 128 * (i // 64) + 64 * h + i % 64
                winseg[c, w] = s
                winpad[c, w] = float(pads[i])
                winreal[c, w] = float(SB - pads[i])
        xts.append(xt)

    # device-owned segments: fully contained in one core's node range
    slots = []
    for c in range(ncores):
        lo, hi = 2 * c * ch, 2 * (c + 1) * ch
        segs = np.unique(xb[lo:hi])
        owned = []
        for s in segs:
            s = int(s)
            first, last = np.searchsorted(xb, [s, s + 1])
            if first >= lo and last <= hi:
                owned.append(s)
        slots.append(owned[:NSLOT])

    # routing matrices R [nwp, NSLOT] with 1/count folded in
    Rs = []
    for c in range(ncores):
        R = np.zeros((nwp, NSLOT), dtype=np.float32)
        slot_of = {s: i for i, s in enumerate(slots[c])}
        for w in range(nwp):
            s = winseg[c, w]
            if s >= 0 and s in slot_of:
                R[w, slot_of[s]] = 1.0 / max(counts[s], 1.0)
        Rs.append(R)

    meta = dict(n_sb=n_sb, cols=cols, nwin=nwin, nch=nch, nwp=nwp,
                counts=counts, winseg=winseg, winpad=winpad, winreal=winreal,
                slots=slots, Rs=Rs, ncores=ncores)
    return xts, meta


def _build(n_sb, nch, ncores=NCORES):
    nc = bacc.Bacc("TRN2", target_bir_lowering=False, debug=False,
                   num_devices=ncores)
    cols = n_sb * SB
    nwp = WPC * nch
    xt_d = nc.dram_tensor("xt", [128, cols], FP8, kind="ExternalInput").ap()
    wza_d = nc.dram_tensor("wza", [128, 128], FP8, kind="ExternalInput").ap()
    wzb_d = nc.dram_tensor("wzb", [128, 128], FP8, kind="ExternalInput").ap()
    b1_d = nc.dram_tensor("b1", [128, 1], F32, kind="ExternalInput").ap()
    nb1_d = nc.dram_tensor("nb1", [128, 1], F32, kind="ExternalInput").ap()
    corr_d = nc.dram_tensor("corr", [128, nwp], F32, kind="ExternalInput").ap()
    g_d = nc.dram_tensor("g", [128, 128], F32, kind="ExternalInput").ap()
    r_d = nc.dram_tensor("r", [128, nch * NSLOT], F32, kind="ExternalInput").ap()
    c3_d = nc.dram_tensor("c3", [128, 1], F32, kind="ExternalInput").ap()
    w4_d = nc.dram_tensor("w4", [128, ODIM], BF16, kind="ExternalInput").ap()
    b4_d = nc.dram_tensor("b4", [ODIM, 1], F32, kind="ExternalInput").ap()
    p_d = nc.dram_tensor("p_out", [128, nwp], F32, kind="ExternalOutput").ap()
    out_d = nc.dram_tensor("out_t", [ODIM, NSLOT], F32, kind="ExternalOutput").ap()

    sizes = _chunk_sizes(n_sb)
    starts = np.cumsum([0] + sizes[:-1])

    with tile.TileContext(nc) as tc:
        with tc.tile_pool(name="const", bufs=1) as cpool, \
             tc.tile_pool(name="xin", bufs=3) as xpool, \
             tc.tile_pool(name="tr", bufs=1) as trpool, \
             tc.tile_pool(name="ps", bufs=2, space="PSUM") as pspool:

            # first x chunk before the constants: PE starts ~1us in
            xtile = xpool.tile([128, sizes[0] * SB], FP8, tag="x")
            nc.sync.dma_start(xtile[:], xt_d[:, 0:sizes[0] * SB])

            wza = cpool.tile([128, 128], FP8)
            nc.sync.dma_start(wza[:], wza_d[:])
            wzb = cpool.tile([128, 128], FP8)
            nc.sync.dma_start(wzb[:], wzb_d[:])
            b1t = cpool.tile([128, 1], F32)
            nc.sync.dma_start(b1t[:], b1_d[:])
            nb1t = cpool.tile([128, 1], F32)
            nc.sync.dma_start(nb1t[:], nb1_d[:])
            corr = cpool.tile([128, nwp], F32)
            nc.sync.dma_start(corr[:], corr_d[:])
            gt = cpool.tile([128, 128], F32)
            nc.sync.dma_start(gt[:], g_d[:])
            rt = cpool.tile([128, nch * NSLOT], F32)
            nc.sync.dma_start(rt[:], r_d[:])
            c3t = cpool.tile([128, 1], F32)
            nc.sync.dma_start(c3t[:], c3_d[:])
            w4t = cpool.tile([128, ODIM], BF16)
            nc.sync.dma_start(w4t[:], w4_d[:])
            b4t = cpool.tile([ODIM, 1], F32)
            nc.sync.dma_start(b4t[:], b4_d[:])

            Pa = [cpool.tile([128, 64], F32, name=f"Pa{k}") for k in range(nch)]
            Pd = [cpool.tile([128, 64], F32, name=f"Pd{k}") for k in range(nch)]
            for Pk in Pa + Pd:
                nc.vector.memset(Pk[:], 0.0)
            Pc = [cpool.tile([128, WPC], F32, name=f"Pc{k}") for k in range(nch)]
            H = [cpool.tile([128, 128], F32, name=f"H{k}") for k in range(nch)]
            trash_d = trpool.tile([128, SB], BF16, tag="td")

            def chunk_tail(k):
                """Pc = P + corr for chunk k (SBUF-only, overlapped
                with the stream) + partials export."""
                sl = slice(WPC * k, WPC * (k + 1))
                nc.vector.tensor_tensor(out=Pc[k][:, 0:64], in0=Pa[k][:],
                                        in1=corr[:, WPC * k:WPC * k + 64],
                                        op=mybir.AluOpType.add)
                nc.vector.tensor_tensor(out=Pc[k][:, 64:128], in0=Pd[k][:],
                                        in1=corr[:, WPC * k + 64:WPC * (k + 1)],
                                        op=mybir.AluOpType.add)
                nc.sync.dma_start(p_d[:, sl], Pc[k][:])

            done_chunks = 0
            ci = 0
            for sb in range(n_sb):
                if ci < len(sizes) and sb == starts[ci]:
                    if ci > 0:
                        xtile = xpool.tile([128, sizes[ci] * SB], FP8, tag="x")
                        lo = starts[ci] * SB
                        nc.sync.dma_start(xtile[:],
                                          xt_d[:, lo:lo + sizes[ci] * SB])
                    ci += 1
                base = (sb - starts[ci - 1]) * SB
                psa = pspool.tile([128, SB], F32, tag="psa")
                psb = pspool.tile([128, SB], F32, tag="psb")
                for t in range(SB // 512):
                    off = base + 512 * t
                    nc.tensor.matmul(
                        psa[:, 512 * t:512 * t + 512], lhsT=wza[:],
                        rhs=xtile[:, off:off + 512], start=True, stop=True)
                for t in range(SB // 512):
                    off = base + 512 * t
                    nc.tensor.matmul(
                        psb[:, 512 * t:512 * t + 512], lhsT=wzb[:],
                        rhs=xtile[:, off:off + 512], start=True, stop=True)
                k, j = sb // 64, sb % 64
                # half A -> ACT, relu written back in place to PSUM
                nc.scalar.activation(
                    out=psa[:], in_=psa[:],
                    func=mybir.ActivationFunctionType.Relu,
                    bias=b1t[:, 0:1],
                    accum_out=Pa[k][:, j:j + 1])
                if _flip(sb):
                    # half B -> ACT too (balance)
                    nc.scalar.activation(
                        out=psb[:], in_=psb[:],
                        func=mybir.ActivationFunctionType.Relu,
                        bias=b1t[:, 0:1],
                        accum_out=Pd[k][:, j:j + 1])
                else:
                    # half B -> DVE
                    nc.vector.tensor_scalar(
                        out=trash_d[:], in0=psb[:],
                        scalar1=nb1t[:, 0:1], scalar2=0.0,
                        op0=mybir.AluOpType.max, op1=mybir.AluOpType.add,
                        accum_out=Pd[k][:, j:j + 1])
                while (done_chunks + 1) * 64 <= sb + 1:
                    chunk_tail(done_chunks)
                    done_chunks += 1
            while done_chunks < nch:
                chunk_tail(done_chunks)
                done_chunks += 1

            # ---- rho tail ----
            T3 = cpool.tile([128, 128], BF16)
            OT = cpool.tile([ODIM, NSLOT], F32)

            for k in range(nch):
                ps1 = pspool.tile([128, SB], F32, tag="psb")
                nc.tensor.matmul(ps1[:, 0:128], lhsT=Pc[k][:], rhs=gt[:],
                                 start=True, stop=True)
                nc.scalar.copy(out=H[k][:], in_=ps1[:, 0:128])

            # psum2 = sum_k H_k.T @ R_k = (mean @ G).T  [t3pre, slot]
            ps2 = pspool.tile([128, SB], F32, tag="psa")
            for k in range(nch):
                nc.tensor.matmul(ps2[:, 0:128], lhsT=H[k][:],
                                 rhs=rt[:, NSLOT * k:NSLOT * k + NSLOT],
                                 start=(k == 0), stop=(k == nch - 1))
            nc.scalar.activation(
                out=T3[:], in_=ps2[:, 0:128],
                func=mybir.ActivationFunctionType.Relu,
                bias=c3t[:, 0:1])

            ps4 = pspool.tile([128, SB], F32, tag="psa")
            nc.tensor.matmul(ps4[0:ODIM, 0:128], lhsT=w4t[:], rhs=T3[:],
                             start=True, stop=True)
            nc.vector.tensor_scalar(
                out=OT[:], in0=ps4[0:ODIM, 0:128],
                scalar1=b4t[:, 0:1], scalar2=None,
                op0=mybir.AluOpType.add)
            nc.sync.dma_start(out_d[:], OT[:])

    nc.compile()
    return nc


def run(inputs, ncores=NCORES, trace=False):
    x = np.asarray(inputs["x"], dtype=np.float32)
    xb = np.asarray(inputs["x_batch"])
    W1 = np.asarray(inputs["W1"], dtype=np.float32)
    b1 = np.asarray(inputs["b1"], dtype=np.float32)
    W2 = np.asarray(inputs["W2"], dtype=np.float64)
    b2 = np.asarray(inputs["b2"], dtype=np.float64)
    W3 = np.asarray(inputs["W3"], dtype=np.float64)
    b3 = np.asarray(inputs["b3"], dtype=np.float64)
    W4 = np.asarray(inputs["W4"], dtype=np.float64)
    b4 = np.asarray(inputs["b4"], dtype=np.float64)

    xts, meta = _host_prep(x, xb, ncores=ncores)
    n_sb, nch, nwp = meta["n_sb"], meta["nch"], meta["nwp"]
    counts = meta["counts"]

    wza = np.zeros((128, 128), dtype=np.float32)
    wza[0:64, :] = W1
    wzb = np.zeros((128, 128), dtype=np.float32)
    wzb[64:128, :] = W1
    b1c = np.ascontiguousarray(b1, np.float32).reshape(128, 1)
    nb1c = np.ascontiguousarray(-b1, np.float32).reshape(128, 1)

    G = np.ascontiguousarray((W2 @ W3).astype(np.float32))
    c3 = np.ascontiguousarray((b2 @ W3 + b3).astype(np.float32)).reshape(128, 1)
    w4c = np.ascontiguousarray(W4.astype(np.float32)).astype(ml_dtypes.bfloat16)
    b4c = np.ascontiguousarray(b4.astype(np.float32)).reshape(ODIM, 1)
    # correction: Pc = P + corr
    #   ACT window w: sum = true + padcnt_w * relu(b1)
    #   DVE window w: sum = true - realcnt_w * b1 + padcnt_w * relu(-b1)
    in_maps = []
    sbs = np.arange(nwp)
    inb = (sbs % WPC) >= 64
    sb_of_w = 64 * (sbs // WPC) + sbs % 64
    flipped = np.array([_flip(int(s)) for s in sb_of_w])
    for c in range(ncores):
        isdve = inb & ~flipped
        pad = meta["winpad"][c]
        real = meta["winreal"][c]
        corr_t = (np.where(isdve, real, 0.0)[None, :] * b1[:, None]
                  + np.where(isdve, -pad, 0.0)[None, :]
                  * np.maximum(-b1, 0.0)[:, None]
                  + np.where(isdve, 0.0, -pad)[None, :]
                  * np.maximum(b1, 0.0)[:, None]).astype(np.float32)
        R = meta["Rs"][c]
        rstack = np.zeros((128, nch * NSLOT), dtype=np.float32)
        for k in range(nch):
            rstack[:, NSLOT * k:NSLOT * (k + 1)] = R[WPC * k:WPC * k + WPC, :]
        in_maps.append(dict(
            xt=xts[c], wza=wza.astype(F8NP), wzb=wzb.astype(F8NP),
            b1=b1c, nb1=nb1c, corr=corr_t, g=G, r=rstack,
            c3=c3, w4=w4c, b4=b4c))

    nc = _build(n_sb, nch, ncores=ncores)
    res = run_bass_kernel_spmd(nc, in_maps, core_ids=list(range(ncores)),
                               trace=trace)

    # ---- host assembly ----
    out = np.zeros((NSEG, ODIM), dtype=np.float32)
    done = np.zeros(NSEG, dtype=bool)
    for c in range(ncores):
        ot = res.results[c]["out_t"]
        for i, s in enumerate(meta["slots"][c]):
            out[s] = ot[:, i]
            done[s] = True

    rest = np.where(~done)[0]
    if len(rest):
        sums = np.zeros((NSEG, 128), dtype=np.float64)
        for c in range(ncores):
            Pc = res.results[c]["p_out"].astype(np.float64)
            ws = meta["winseg"][c]
            valid = ws >= 0
            np.add.at(sums, ws[valid], Pc.T[valid])
        need = rest[counts[rest] > 0]
        if len(need):
            mean = sums[need] / counts[need][:, None]
            hid = mean @ W2 + b2
            t3 = np.maximum(hid @ W3 + b3, 0.0)
            out[need] = (t3 @ W4 + b4).astype(np.float32)
        empty = rest[counts[rest] == 0]
        if len(empty):
            row = (np.maximum(b3, 0.0) @ W4 + b4).astype(np.float32)
            out[empty] = row
    return out, res, None


def kernel(**inputs):
    inputs = {k: np.asarray(v) for k, v in inputs.items()}
    out, _, _ = run(inputs)
    return out


if __name__ == "__main__":
    rng = np.random.default_rng(0)
    N, D, HN, B = 8 * 24 * SB, 64, 128, 256
    x = rng.standard_normal((N, D), dtype=np.float32)
    xb = np.sort(rng.integers(0, B, N).astype(np.int32))
    W1 = (rng.standard_normal((D, HN)) / 8).astype(np.float32)
    W2 = (rng.standard_normal((HN, HN)) / 11.3).astype(np.float32)
    W3 = (rng.standard_normal((HN, HN)) / 11.3).astype(np.float32)
    W4 = (rng.standard_normal((HN, ODIM)) / 11.3).astype(np.float32)
    b1 = rng.standard_normal(HN).astype(np.float32) * 0.1
    b2 = rng.standard_normal(HN).astype(np.float32) * 0.1
    b3 = rng.standard_normal(HN).astype(np.float32) * 0.1
    b4 = rng.standard_normal(ODIM).astype(np.float32) * 0.1
    ins = dict(x=x, x_batch=xb, W1=W1, b1=b1, W2=W2, b2=b2, W3=W3, b3=b3,
               W4=W4, b4=b4)
    out = kernel(**ins)

    h = np.maximum(x @ W1 + b1, 0) @ W2 + b2
    sums = np.zeros((1024, HN), dtype=np.float64)
    np.add.at(sums, xb, h.astype(np.float64))
    cnt = np.bincount(xb, minlength=1024).astype(np.float64)
    mean = sums / np.maximum(cnt, 1)[:, None]
    ref = (np.maximum(mean @ W3 + b3, 0) @ W4 + b4).astype(np.float32)
    num = np.linalg.norm(out - ref)
    den = np.linalg.norm(ref)
    print("Relative error:", num / den)
